# revision 18
# baseline (speedup 1.0000x reference)
"""Trainium2 Bass kernel for DGCRNNCell (nn_DGCRNNCell_21792664060192).

Computes, for each batch item b and head h over graph with N=199 nodes:
  feat   = einsum('nf,nm->mf', X[b], A*W[h])          (via featT chain)
  dense  = feat @ kernel[h] + bias1[h]
  mask   = softmax(dense - NEG*(1-A), axis=-1)        (adjacency-masked softmax)
  node   = mask @ X[b]
  out_h  = node @ T[h] + bias2[h]
  output[b] = concat([out_0..out_3 (r, 256)], mask_3 (r, 199))   -> (199, 455)

Sharding: pure data-parallel over batch (512 -> 64 per core x 8 cores).

v3 dataflow (per core), built around item PAIRS and engine balance
(GPSIMD cannot touch PSUM on TRN2, so all PSUM-side elementwise work is
split between ACT and DVE with as few, as large instructions as possible):
  step1  featT for a pair (b0,b1): lhsT = [X[b0] | X[b1]] (cn, 128) so the
         pair's f-rows land on partitions 0-63 / 64-127; rhs = AW head-pair
         (cn, 2*199).  4 matmuls of free 398 per pair.
  fs     PSUM->SBUF bf16 copy of the pair's featT; alternates ACT/DVE.
  dense  per item: adjacency mask + bias1 written via an fp8 DoubleRow
         identity matmul (half cycles; -60/0 are exact in fp8e4), then 8
         bf16 matmuls accumulate kernel[h]^T @ featT; exp on ACT gives the
         masked e directly (one activation per c-chunk).
  XT     per item: X_aug @ T_aug, ONE matmul per c-chunk (free 260);
         col 64 of each head block = ones -> s; TA row 64 = bias2.
  step5  out = (e_h)^T @ XT_h accumulated over c-chunks into a 2-item PSUM
         tile; head-3 mask via PE transpose of e3 into a 2-item PSUM tile.
  stage  per item pair: ONE reciprocal, ONE normalize-multiply (512 free)
         and ONE mask3-multiply (398 free) on DVE; output staged bf16
         (host casts to fp32).
PSUM budget (8 banks): ring{fAB,d0,d1,XT} 2x2 banks; oUF2 2 banks;
pR2 2x1 banks.
"""

import numpy as np

import concourse.bass as bass
import concourse.mybir as mybir
import concourse.tile as tile
from concourse import bacc

B, N, F, U, H = 512, 199, 64, 64, 4
NCORES = 8
BPC = B // NCORES  # 64 batch items per core
P0 = 128
P1 = N - P0  # 71
FA = F + 1  # X augmented with ones column (XaT row 64 = ones)
OUTC = H * U + N  # 455
DT = mybir.dt.float32
BF = mybir.dt.bfloat16
F8 = mybir.dt.float8e4
AF = mybir.ActivationFunctionType
ALU = mybir.AluOpType
PM = mybir.MatmulPerfMode

_CHUNKS = ((0, P0), (P0, P1))  # (offset, size) along the N(=c or r) axis


def _build_kernel_v3(nc: bass.Bass, tc: "tile.TileContext", io: dict, bpc: int = BPC):
    import os
    from contextlib import ExitStack

    Xf, XaT, AWp, K2, MK8, ID8, TA, ID, O = (
        io["Xf"], io["XaT"], io["AWp"], io["K2"], io["MK8"], io["ID8"],
        io["TA"], io["ID"], io["O"],
    )

    def _b(name, default):
        return int(os.environ.get(name, str(default)))

    fse = os.environ.get("FSE", "alt")     # fs copy engine: alt|scalar|vector
    mask8 = os.environ.get("MASK8", "1") == "1"  # fp8 DoubleRow mask write

    with ExitStack() as ctx:
        cpool = ctx.enter_context(tc.tile_pool(name="consts", bufs=1))
        xpool = ctx.enter_context(tc.tile_pool(name="xf", bufs=_b("XB", 2)))
        fspool = ctx.enter_context(tc.tile_pool(name="fs", bufs=_b("FSB", 2)))
        epool = ctx.enter_context(tc.tile_pool(name="expT", bufs=_b("EB", 3)))
        rpool = ctx.enter_context(tc.tile_pool(name="rec", bufs=_b("RB", 4)))
        opool = ctx.enter_context(tc.tile_pool(name="sO", bufs=_b("OB", 2)))

        # ---- constants into SBUF (once) ----
        cAW = []
        cMK = []
        for ci, (co, cn) in enumerate(_CHUNKS):
            t = cpool.tile([cn, 2, 2 * N], BF, name=f"cAW{ci}")
            nc.sync.dma_start(t[:], AWp[co : co + cn])
            cAW.append(t)
            if mask8:
                t = cpool.tile([cn, 2, 2, 2 * N], F8, name=f"cMK{ci}")
                nc.sync.dma_start(t[:], MK8[co : co + cn])
                cMK.append(t)
        cK2 = cpool.tile([128, H, N], BF, name="cK2")
        cTA = cpool.tile([FA, H, FA], BF, name="cTA")
        cID = cpool.tile([128, 128], BF, name="cID")
        nc.sync.dma_start(cK2[:], K2[:])
        nc.sync.dma_start(cTA[:], TA[:])
        nc.sync.dma_start(cID[:], ID[:])
        if mask8:
            cID8 = cpool.tile([128, 2, 128], F8, name="cID8")
            nc.sync.dma_start(cID8[:], ID8[:])

        BG = min(_b("BG", 8), bpc)   # input DMA batching
        OG = min(_b("OG", 4), bpc)   # output DMA batching (multiple of 2)

        # ---- prologue: XT = Xa_aug @ TA_aug for ALL items, kept in SBUF ----
        # cXTall[c, b, cc, 65h + j]: cols 0-63 of each head block = XT data,
        # col 64 = ones (the softmax-denominator column), written once.
        cXTall = cpool.tile([128, bpc, 2, H * FA], BF, name="cXTall")
        nc.vector.memset(
            cXTall[:].rearrange("p b c (h j) -> p b c h j", j=FA)[:, :, :, :, 64],
            1.0,
        )
        PG = 4  # items per prologue PSUM tile
        with tc.tile_pool(name="pxt", bufs=_b("XTB", 2), space="PSUM") as pxt:
            for b0 in range(0, bpc, PG):
                if b0 % BG == 0:
                    ng = min(BG, bpc - b0)
                    xtg = xpool.tile([FA, BG * N], BF, tag="xat")
                    nc.sync.dma_start(
                        xtg[:, 0 : ng * N].rearrange("j (g n) -> j g n", n=N),
                        XaT[b0 : b0 + ng].rearrange("g j n -> j g n"),
                    )
                gi = b0 % BG
                XTp = pxt.tile([128, PG, 2, 256], DT, tag="xtp")
                for g in range(PG):
                    xt = xtg[:, (gi + g) * N : (gi + g + 1) * N]
                    for ci, (co, cn) in enumerate(_CHUNKS):
                        nc.tensor.matmul(
                            XTp[0:cn, g, ci, :],
                            lhsT=xt[:, co : co + cn],
                            rhs=cTA[:, :, 0:U],
                            start=True,
                            stop=True,
                        )
                dst = cXTall[:, b0 : b0 + PG].rearrange(
                    "p b c (h j) -> p b c h j", j=FA
                )[:, :, :, :, 0:U]
                if (b0 // PG) % 2 == 0:
                    nc.scalar.copy(
                        dst, XTp[:].rearrange("p b c (h u) -> p b c h u", u=U)
                    )
                else:
                    nc.vector.tensor_copy(
                        dst, XTp[:].rearrange("p b c (h u) -> p b c h u", u=U)
                    )

        pd = ctx.enter_context(
            tc.tile_pool(name="pdnu", bufs=_b("DTB", 2), space="PSUM")
        )
        po = ctx.enter_context(
            tc.tile_pool(name="poU", bufs=_b("POB", 2), space="PSUM")
        )

        def load_xgroup(b0):
            ng = min(BG, bpc - b0)
            src = Xf[b0 : b0 + ng].rearrange("g n f -> n g f")
            xg = []
            for ci, (co, cn) in enumerate(_CHUNKS):
                t = xpool.tile([cn, BG, F], BF, tag=f"xf{ci}")
                nc.sync.dma_start(t[:, 0:ng, :], src[co : co + cn])
                xg.append(t)
            return xg

        def step1(xg, b0, q):
            # step1 for a pair: out partitions = [b0 f | b1 f].  Allocated
            # from the dnu ring; emitted one pair AHEAD so the copy+dense of
            # pair q+1 overlap the exp drain of pair q.
            gi = b0 % BG
            fAB = pd.tile([128, 2, 512], DT, tag="dnu", name="fAB")
            for hp in range(2):
                for ci, (co, cn) in enumerate(_CHUNKS):
                    nc.tensor.matmul(
                        fAB[:, hp, 0 : 2 * N],
                        lhsT=xg[ci][:, gi : gi + 2, :],
                        rhs=cAW[ci][:, hp, :],
                        start=(ci == 0),
                        stop=(ci == 1),
                    )
            fs = fspool.tile([128, 2, 2 * N], BF, tag="fs")
            if fse == "split":
                nc.scalar.copy(fs[:, 0], fAB[:, 0, 0 : 2 * N])
                nc.vector.tensor_copy(fs[:, 1], fAB[:, 1, 0 : 2 * N])
            elif fse == "scalar" or (fse == "alt" and q % 2 == 0):
                nc.scalar.copy(fs[:], fAB[:, :, 0 : 2 * N])
            else:
                nc.vector.tensor_copy(fs[:], fAB[:, :, 0 : 2 * N])
            return fs

        xg = load_xgroup(0)
        fs_next = step1(xg, 0, 0)
        sog = [None, None]
        for q in range(bpc // 2):
            b0 = 2 * q
            fs = fs_next

            def dense_chunk(g, ci, pool):
                # dense chunk + exp; head h -> slot s=h%2, block k=h//2
                co, cn = _CHUNKS[ci]
                d = pool.tile([128, 2, 512], DT,
                              tag="dnu" if pool is pd else "oU",
                              name=f"dT{g}{ci}")
                if mask8:
                    for s in range(2):
                        nc.tensor.matmul(
                            d[0:cn, s, 0 : 2 * N],
                            lhsT=cID8[0:cn, :, 0:cn],
                            rhs=cMK[ci][:, s],
                            start=True,
                            stop=False,
                            perf_mode=PM.DoubleRow,
                        )
                for h in range(H):
                    nc.tensor.matmul(
                        d[0:cn, h % 2, 199 * (h // 2) : 199 * (h // 2) + N],
                        lhsT=cK2[64 * g : 64 * g + 64, h, co : co + cn],
                        rhs=fs[64 * g : 64 * g + 64, h // 2,
                               199 * (h % 2) : 199 * (h % 2) + N],
                        start=not mask8,
                        stop=True,
                        tile_position=(64 * g, 0),
                    )
                e = epool.tile([cn, 2, 2 * N], BF, tag=f"eT{ci}")
                nc.scalar.activation(e[:], d[0:cn, :, 0 : 2 * N], AF.Exp)
                return e

            go = b0 % OG
            if go == 0:
                sog = [
                    opool.tile([rn, OG, OUTC], BF, tag=f"sO{ci}", name=f"sOg{ci}")
                    for ci, (ro, rn) in enumerate(_CHUNKS)
                ]

            # item 0's dense through the dnu ring, item 1's through the oU
            # ring: the next pair's step1 then reuses the buffer freed by the
            # FIRST exp, so it overlaps the tail of this pair's exp drain.
            dp1 = po if os.environ.get("D1P", "0") == "1" else pd
            eB0 = [dense_chunk(0, 0, pd), dense_chunk(0, 1, pd)]
            eB1 = [dense_chunk(1, 0, dp1), dense_chunk(1, 1, dp1)]
            eTg = [eB0, eB1]

            oU2 = [po.tile([128, 2, 512], DT, tag="oU", name=f"oUF{ci}")
                   for ci in range(2)]
            # head-3 mask lives (as bf16) in the padding of the oU2 slots:
            # slot bytes [0:1040) hold the 260-col step5 output, [1040:1840)
            # hold the transposed e3 row-chunk for the same r-range.
            pRv = [oU2[ci][:].bitcast(BF) for ci in range(2)]

            def transposes(g, eT):
                # head-3 mask transposed into (r, c): PE-transpose of e3
                for rj, (ro, rn) in enumerate(_CHUNKS):
                    for ci, (co, cn) in enumerate(_CHUNKS):
                        nc.tensor.transpose(
                            pRv[rj][0:rn, g, 520 + co : 520 + co + cn],
                            in_=eT[ci][:, 1, N + ro : N + ro + rn],
                            identity=cID[0:cn, 0:cn],
                        )

            def step5(g, ci, eT):
                ro, rn = _CHUNKS[ci]
                for h in range(H):
                    for cc, (co, cn) in enumerate(_CHUNKS):
                        nc.tensor.matmul(
                            oU2[ci][0:rn, g, 65 * h : 65 * h + 65],
                            lhsT=eT[cc][
                                :, h % 2,
                                199 * (h // 2) + ro : 199 * (h // 2) + ro + rn,
                            ],
                            rhs=cXTall[0:cn, b0 + g, cc, 65 * h : 65 * h + 65],
                            start=(cc == 0),
                            stop=(cc == 1),
                        )

            transposes(0, eB0)
            step5(0, 0, eB0)
            step5(0, 1, eB0)
            # pipelined step1 for the NEXT pair sits here: PE-independent of
            # this pair's remaining exps, fills the wait for exp(b1).
            if b0 + 2 < bpc:
                if (b0 + 2) % BG == 0:
                    xg = load_xgroup(b0 + 2)
                fs_next = step1(xg, b0 + 2, q + 1)
            transposes(1, eB1)
            step5(1, 0, eB1)
            step5(1, 1, eB1)

            for ci, (ro, rn) in enumerate(_CHUNKS):
                oUF2 = oU2[ci]
                # 1/s for both items x 4 heads: s at col 64 of each 65-block
                rec = rpool.tile([rn, 2, H], DT, tag=f"rec{ci}")
                oUh = oUF2[0:rn, :, 0 : H * FA].rearrange(
                    "p g (h j) -> p g h j", j=FA
                )
                nc.vector.reciprocal(rec[:], oUh[:, :, :, 64])

                sO2 = sog[ci][:, go : go + 2]
                nc.vector.tensor_tensor(
                    sO2[:, :, 0 : H * U].rearrange("p g (h u) -> p g h u", u=U),
                    oUh[:, :, :, 0:U],
                    rec[:, :, :, None].to_broadcast((rn, 2, H, U)),
                    ALU.mult,
                )
                # mask3 normalize as per-item ACT scale-copies: these have no
                # fan-in until the pair's very end, so they execute in the
                # ACT gap at the next pair's start (exp waits on dense there).
                if os.environ.get("M3E", "vector") == "scalar":
                    for g in range(2):
                        nc.scalar.activation(
                            sO2[:, g, H * U : OUTC],
                            pRv[ci][0:rn, g, 520 : 520 + N],
                            AF.Copy,
                            scale=rec[:, g, 3:4],
                        )
                else:
                    nc.vector.tensor_tensor(
                        sO2[:, :, H * U : OUTC],
                        pRv[ci][0:rn, :, 520 : 520 + N],
                        rec[:, :, 3:4].to_broadcast((rn, 2, N)),
                        ALU.mult,
                    )

                if go + 2 == OG or b0 + 2 >= bpc:
                    ngo = go + 2
                    nc.sync.dma_start(
                        O[b0 + 1 - (ngo - 1) : b0 + 2, ro : ro + rn].rearrange(
                            "g n c -> n g c"
                        ),
                        sog[ci][:, 0:ngo],
                    )


def build_nc(
    bpc: int = BPC, num_devices: int = NCORES, repeat: int = 1
) -> bass.Bass:
    nc = bacc.Bacc(
        "TRN2",
        target_bir_lowering=False,
        debug=False,
        num_devices=num_devices,
    )
    io = {
        "Xf": nc.dram_tensor("Xf", [bpc, N, F], BF, kind="ExternalInput").ap(),
        "XaT": nc.dram_tensor("XaT", [bpc, FA, N], BF, kind="ExternalInput").ap(),
        "AWp": nc.dram_tensor("AWp", [N, 2, 2 * N], BF, kind="ExternalInput").ap(),
        "K2": nc.dram_tensor("K2", [128, H, N], BF, kind="ExternalInput").ap(),
        "MK8": nc.dram_tensor("MK8", [N, 2, 2, 2 * N], F8, kind="ExternalInput").ap(),
        "ID8": nc.dram_tensor("ID8", [128, 2, 128], F8, kind="ExternalInput").ap(),
        "TA": nc.dram_tensor("TA", [FA, H, FA], BF, kind="ExternalInput").ap(),
        "ID": nc.dram_tensor("ID", [128, 128], BF, kind="ExternalInput").ap(),
        "O": nc.dram_tensor("O", [bpc, N, OUTC], BF, kind="ExternalOutput").ap(),
    }
    with tile.TileContext(nc) as tc:
        if repeat == 1:
            _build_kernel_v3(nc, tc, io, bpc=bpc)
        else:
            # Timing-only variant: re-run the identical workload `repeat`
            # times in a hardware loop so per-dispatch tunnel latency can be
            # amortized out of the hardware-time measurement.
            import os as _os

            if _os.environ.get("STAGR", "1") == "1":
                with tc.For_i(0, repeat, 1, staggered_reset=True):
                    _build_kernel_v3(nc, tc, io, bpc=bpc)
            else:
                with tc.For_i(0, repeat, 1):
                    _build_kernel_v3(nc, tc, io, bpc=bpc)
    nc.compile()
    return nc


def _prep_weights(A, W, kernel, T, bias1, bias2):
    """Host-side constant prep (tiny tensors)."""
    A = np.asarray(A, np.float32)
    W = np.asarray(W, np.float32)
    kernel = np.asarray(kernel, np.float32)
    T = np.asarray(T, np.float32)
    bias1 = np.asarray(bias1, np.float32)
    bias2 = np.asarray(bias2, np.float32)

    AW = A[None, :, :] * W  # (H, n, m)
    # AWp[n, hp, k*199+m] = AW[2hp+k][n, m]
    AWp = np.ascontiguousarray(
        AW.reshape(2, 2, N, N).transpose(2, 0, 1, 3).reshape(N, 2, 2 * N)
    )

    Kf = kernel  # (H, F, N): [h, f, c]
    K1 = np.ascontiguousarray(Kf.transpose(1, 0, 2))  # [f, h, c]
    K2 = np.concatenate([K1, K1], axis=0)  # duplicate f-rows for PE rows 64-127

    # MK[c, h, m] = bias1[h, c] - 60 * (1 - A[m, c]): additive logit fixup
    # (adjacency mask + bias1); -60 and 0 are exactly representable in fp8e4.
    # Packed as [c, s, ktile, k*199 + m] with h = 2k + s; ktile 1 is zeros
    # (the second DoubleRow contraction tile contributes nothing).
    MK = bias1.T[:, :, None] - 60.0 * (1.0 - A.T[:, None, :])  # (c, h, m)
    MKs = MK.reshape(N, 2, 2, N).transpose(0, 2, 1, 3).reshape(N, 2, 2 * N)
    MK8 = np.zeros((N, 2, 2, 2 * N), np.float32)
    MK8[:, :, 0, :] = MKs

    # T_aug[h]: (65, 65): rows 0-63 = T[h], row 64 = [bias2[h], 1.0-at-col-64]
    TA = np.zeros((FA, H, FA), np.float32)
    TA[:F, :, :U] = T.transpose(1, 0, 2)
    TA[F, :, :U] = bias2
    TA[F, :, U] = 1.0

    ID8 = np.zeros((128, 2, 128), np.float32)
    ID8[:, 0, :] = np.eye(128, dtype=np.float32)

    import ml_dtypes

    bf = ml_dtypes.bfloat16
    f8 = ml_dtypes.float8_e4m3
    return dict(
        AWp=AWp.astype(bf), K2=K2.astype(bf), MK8=MK8.astype(f8),
        ID8=ID8.astype(f8), TA=TA.astype(bf), ID=np.eye(128, dtype=bf),
    )


_CACHED = {}


def _get_executable(repeat: int = 1):
    """Build the Bass module once and wrap it in a reusable sharded jax jit.

    Mirrors concourse.bass2jax.run_bass_via_pjrt's multi-core path, but caches
    the jitted callable so repeated kernel() calls skip re-lowering the BIR.
    """
    if repeat in _CACHED:
        return _CACHED[repeat]

    import jax
    from jax.sharding import Mesh, PartitionSpec
    from jax.experimental.shard_map import shard_map

    import concourse.mybir as _mybir
    from concourse import bass2jax

    bass2jax.install_neuronx_cc_hook()
    nc = build_nc(repeat=repeat)

    partition_name = (
        nc.partition_id_tensor.name if nc.partition_id_tensor else None
    )
    in_names, out_names, out_avals = [], [], []
    for alloc in nc.m.functions[0].allocations:
        if not isinstance(alloc, _mybir.MemoryLocationSet):
            continue
        name = alloc.memorylocations[0].name
        if alloc.kind == "ExternalInput":
            if name != partition_name:
                in_names.append(name)
        elif alloc.kind == "ExternalOutput":
            out_names.append(name)
            out_avals.append(
                jax.core.ShapedArray(
                    tuple(alloc.tensor_shape), _mybir.dt.np(alloc.dtype)
                )
            )
    n_params = len(in_names)
    n_outs = len(out_avals)
    all_in_names = list(in_names) + list(out_names)
    if partition_name is not None:
        all_in_names.append(partition_name)

    def _body(*args):
        operands = list(args)
        if partition_name is not None:
            operands.append(bass2jax.partition_id_tensor())
        outs = bass2jax._bass_exec_p.bind(
            *operands,
            out_avals=tuple(out_avals),
            in_names=tuple(all_in_names),
            out_names=tuple(out_names),
            lowering_input_output_aliases=(),
            sim_require_finite=True,
            sim_require_nnan=True,
            nc=nc,
        )
        return tuple(outs)

    devices = jax.devices()[:NCORES]
    mesh = Mesh(np.asarray(devices), ("core",))
    in_specs = (PartitionSpec("core"),) * (n_params + n_outs)
    out_specs = (PartitionSpec("core"),) * n_outs
    sharded = jax.jit(
        shard_map(
            _body, mesh=mesh, in_specs=in_specs, out_specs=out_specs,
            check_rep=False,
        ),
        donate_argnums=tuple(range(n_params, n_params + n_outs)),
        keep_unused=True,
    )
    _CACHED[repeat] = (sharded, in_names, out_names, out_avals, jax, mesh)
    return _CACHED[repeat]


def _stage_inputs(inputs):
    import ml_dtypes

    X = np.asarray(inputs["X"], np.float32)
    consts = _prep_weights(
        inputs["A"], inputs["W"], inputs["kernel"], inputs["T"],
        inputs["bias1"], inputs["bias2"],
    )
    bf = ml_dtypes.bfloat16
    Xb = X.astype(bf)
    XaT = np.concatenate(
        [X.transpose(0, 2, 1), np.ones((B, 1, N), np.float32)], axis=1
    ).astype(bf)
    per_core = {
        "Xf": np.ascontiguousarray(Xb),
        "XaT": np.ascontiguousarray(XaT),
    }
    for k, v in consts.items():
        per_core[k] = np.concatenate([v] * NCORES, axis=0)
    return per_core


def _run(staged):
    sharded, in_names, out_names, out_avals, jax, mesh = _get_executable()
    concat_in = [staged[nm] for nm in in_names]
    zeros = [
        np.zeros((NCORES * a.shape[0], *a.shape[1:]), a.dtype) for a in out_avals
    ]
    out_arrs = sharded(*concat_in, *zeros)
    return np.asarray(out_arrs[out_names.index("O")])


def kernel(**inputs) -> np.ndarray:
    staged = _stage_inputs(inputs)
    out = _run(staged)  # (NCORES*BPC, N, OUTC) = (B, N, OUTC) bf16
    return out.astype(np.float32)


# revision 23
# speedup vs baseline: 1.0158x; 1.0158x over previous
"""Trainium2 Bass kernel for DGCRNNCell (nn_DGCRNNCell_21792664060192).

Computes, for each batch item b and head h over graph with N=199 nodes:
  feat   = einsum('nf,nm->mf', X[b], A*W[h])          (via featT chain)
  dense  = feat @ kernel[h] + bias1[h]
  mask   = softmax(dense - NEG*(1-A), axis=-1)        (adjacency-masked softmax)
  node   = mask @ X[b]
  out_h  = node @ T[h] + bias2[h]
  output[b] = concat([out_0..out_3 (r, 256)], mask_3 (r, 199))   -> (199, 455)

Sharding: pure data-parallel over batch (512 -> 64 per core x 8 cores).

v3 dataflow (per core), built around item PAIRS and engine balance
(GPSIMD cannot touch PSUM on TRN2, so all PSUM-side elementwise work is
split between ACT and DVE with as few, as large instructions as possible):
  step1  featT for a pair (b0,b1): lhsT = [X[b0] | X[b1]] (cn, 128) so the
         pair's f-rows land on partitions 0-63 / 64-127; rhs = AW head-pair
         (cn, 2*199).  4 matmuls of free 398 per pair.
  fs     PSUM->SBUF bf16 copy of the pair's featT; alternates ACT/DVE.
  dense  per item: adjacency mask + bias1 written via an fp8 DoubleRow
         identity matmul (half cycles; -60/0 are exact in fp8e4), then 8
         bf16 matmuls accumulate kernel[h]^T @ featT; exp on ACT gives the
         masked e directly (one activation per c-chunk).
  XT     per item: X_aug @ T_aug, ONE matmul per c-chunk (free 260);
         col 64 of each head block = ones -> s; TA row 64 = bias2.
  step5  out = (e_h)^T @ XT_h accumulated over c-chunks into a 2-item PSUM
         tile; head-3 mask via PE transpose of e3 into a 2-item PSUM tile.
  stage  per item pair: ONE reciprocal, ONE normalize-multiply (512 free)
         and ONE mask3-multiply (398 free) on DVE; output staged bf16
         (host casts to fp32).
PSUM budget (8 banks): ring{fAB,d0,d1,XT} 2x2 banks; oUF2 2 banks;
pR2 2x1 banks.
"""

import os as _os

import numpy as np

import concourse.bass as bass
import concourse.mybir as mybir
import concourse.tile as tile
from concourse import bacc

if _os.environ.get("LDWOPT", "0") == "1":
    # The PE spends real time reloading stationary weights before every
    # matmul; walrus's ldweights-dedup optimization (its own default) is
    # pinned off by bass's production caller — turn it back on for this
    # kernel's compile.
    import concourse.bass_utils as _bu

    if not getattr(_bu, "_ldwopt_patched", False):
        _orig_run_command = _bu.run_command

        def _run_command_ldwopt(argv, **kwargs):
            argv = [
                "--enable-ldw-opt=true" if a == "--enable-ldw-opt=false" else a
                for a in argv
            ]
            return _orig_run_command(argv, **kwargs)

        _bu.run_command = _run_command_ldwopt
        _bu._ldwopt_patched = True

B, N, F, U, H = 512, 199, 64, 64, 4
NCORES = 8
BPC = B // NCORES  # 64 batch items per core
P0 = 128
P1 = N - P0  # 71
FA = F + 1  # X augmented with ones column (XaT row 64 = ones)
OUTC = H * U + N  # 455
DT = mybir.dt.float32
BF = mybir.dt.bfloat16
F8 = mybir.dt.float8e4
AF = mybir.ActivationFunctionType
ALU = mybir.AluOpType
PM = mybir.MatmulPerfMode

_CHUNKS = ((0, P0), (P0, P1))  # (offset, size) along the N(=c or r) axis
OGRP = 4  # output DMA item-group (must match the OG default below)


def _build_kernel_v3(nc: bass.Bass, tc: "tile.TileContext", io: dict, bpc: int = BPC):
    import os
    from contextlib import ExitStack

    Xf, XaT, AWp, K2, MK8, ID8, TA, ID, O = (
        io["Xf"], io["XaT"], io["AWp"], io["K2"], io["MK8"], io["ID8"],
        io["TA"], io["ID"], io["O"],
    )

    def _b(name, default):
        return int(os.environ.get(name, str(default)))

    fse = os.environ.get("FSE", "alt")     # fs copy engine: alt|scalar|vector
    mask8 = os.environ.get("MASK8", "1") == "1"  # fp8 DoubleRow mask write

    with ExitStack() as ctx:
        cpool = ctx.enter_context(tc.tile_pool(name="consts", bufs=1))
        xpool = ctx.enter_context(tc.tile_pool(name="xf", bufs=_b("XB", 2)))
        fspool = ctx.enter_context(tc.tile_pool(name="fs", bufs=_b("FSB", 2)))
        epool = ctx.enter_context(tc.tile_pool(name="expT", bufs=_b("EB", 3)))
        rpool = ctx.enter_context(tc.tile_pool(name="rec", bufs=_b("RB", 4)))
        opool = ctx.enter_context(tc.tile_pool(name="sO", bufs=_b("OB", 2)))

        # ---- constants into SBUF (once) ----
        cAW = []
        cMK = []
        for ci, (co, cn) in enumerate(_CHUNKS):
            t = cpool.tile([cn, 2, 2 * N], BF, name=f"cAW{ci}")
            nc.sync.dma_start(t[:], AWp[co : co + cn])
            cAW.append(t)
            if mask8:
                t = cpool.tile([cn, 2, 2, 2 * N], F8, name=f"cMK{ci}")
                nc.sync.dma_start(t[:], MK8[co : co + cn])
                cMK.append(t)
        cK2 = cpool.tile([128, H, N], BF, name="cK2")
        cTA = cpool.tile([FA, H, FA], BF, name="cTA")
        cID = cpool.tile([128, 128], BF, name="cID")
        nc.sync.dma_start(cK2[:], K2[:])
        nc.sync.dma_start(cTA[:], TA[:])
        nc.sync.dma_start(cID[:], ID[:])
        if mask8:
            cID8 = cpool.tile([128, 2, 128], F8, name="cID8")
            nc.sync.dma_start(cID8[:], ID8[:])

        BG = min(_b("BG", 8), bpc)   # input DMA batching
        OG = OGRP  # output DMA batching (matches the DRAM layout)

        # ---- prologue: XT = Xa_aug @ TA_aug for ALL items, kept in SBUF ----
        # cXTall[c, b, cc, 65h + j]: cols 0-63 of each head block = XT data,
        # col 64 = ones (the softmax-denominator column), written once.
        cXTall = cpool.tile([128, bpc, 2, H * FA], BF, name="cXTall")
        nc.vector.memset(
            cXTall[:].rearrange("p b c (h j) -> p b c h j", j=FA)[:, :, :, :, 64],
            1.0,
        )
        PG = 4  # items per prologue PSUM tile
        with tc.tile_pool(name="pxt", bufs=_b("XTB", 2), space="PSUM") as pxt:
            for b0 in range(0, bpc, PG):
                if b0 % BG == 0:
                    ng = min(BG, bpc - b0)
                    xtg = xpool.tile([FA, BG * N], BF, tag="xat")
                    nc.sync.dma_start(
                        xtg[:, 0 : ng * N].rearrange("j (g n) -> j g n", n=N),
                        XaT[b0 : b0 + ng].rearrange("g j n -> j g n"),
                    )
                gi = b0 % BG
                XTp = pxt.tile([128, PG, 2, 256], DT, tag="xtp")
                for g in range(PG):
                    xt = xtg[:, (gi + g) * N : (gi + g + 1) * N]
                    for ci, (co, cn) in enumerate(_CHUNKS):
                        nc.tensor.matmul(
                            XTp[0:cn, g, ci, :],
                            lhsT=xt[:, co : co + cn],
                            rhs=cTA[:, :, 0:U],
                            start=True,
                            stop=True,
                        )
                dst = cXTall[:, b0 : b0 + PG].rearrange(
                    "p b c (h j) -> p b c h j", j=FA
                )[:, :, :, :, 0:U]
                if (b0 // PG) % 2 == 0:
                    nc.scalar.copy(
                        dst, XTp[:].rearrange("p b c (h u) -> p b c h u", u=U)
                    )
                else:
                    nc.vector.tensor_copy(
                        dst, XTp[:].rearrange("p b c (h u) -> p b c h u", u=U)
                    )

        pd = ctx.enter_context(
            tc.tile_pool(name="pdnu", bufs=_b("DTB", 2), space="PSUM")
        )
        po = ctx.enter_context(
            tc.tile_pool(name="poU", bufs=_b("POB", 2), space="PSUM")
        )

        def load_xgroup(b0):
            ng = min(BG, bpc - b0)
            src = Xf[b0 : b0 + ng].rearrange("g n f -> n g f")
            xg = []
            for ci, (co, cn) in enumerate(_CHUNKS):
                t = xpool.tile([cn, BG, F], BF, tag=f"xf{ci}")
                nc.sync.dma_start(t[:, 0:ng, :], src[co : co + cn])
                xg.append(t)
            return xg

        def step1(xg, b0, q):
            # step1 for a pair: out partitions = [b0 f | b1 f].  Allocated
            # from the dnu ring; emitted one pair AHEAD so the copy+dense of
            # pair q+1 overlap the exp drain of pair q.
            gi = b0 % BG
            fAB = pd.tile([128, 2, 512], DT, tag="dnu", name="fAB")
            for hp in range(2):
                for ci, (co, cn) in enumerate(_CHUNKS):
                    nc.tensor.matmul(
                        fAB[:, hp, 0 : 2 * N],
                        lhsT=xg[ci][:, gi : gi + 2, :],
                        rhs=cAW[ci][:, hp, :],
                        start=(ci == 0),
                        stop=(ci == 1),
                    )
            fs = fspool.tile([128, 2, 2 * N], BF, tag="fs")
            if fse == "split":
                nc.scalar.copy(fs[:, 0], fAB[:, 0, 0 : 2 * N])
                nc.vector.tensor_copy(fs[:, 1], fAB[:, 1, 0 : 2 * N])
            elif fse == "scalar" or (fse == "alt" and q % 2 == 0):
                nc.scalar.copy(fs[:], fAB[:, :, 0 : 2 * N])
            else:
                nc.vector.tensor_copy(fs[:], fAB[:, :, 0 : 2 * N])
            return fs

        xg = load_xgroup(0)
        fs_next = step1(xg, 0, 0)
        sog = [None, None]
        for q in range(bpc // 2):
            b0 = 2 * q
            fs = fs_next

            def dense_chunk(g, ci, pool):
                # dense chunk + exp; head h -> slot s=h%2, block k=h//2
                co, cn = _CHUNKS[ci]
                d = pool.tile([128, 2, 512], DT,
                              tag="dnu" if pool is pd else "oU",
                              name=f"dT{g}{ci}")
                if mask8 and os.environ.get("MASKOFF", "0") != "1":
                    for s in range(2):
                        nc.tensor.matmul(
                            d[0:cn, s, 0 : 2 * N],
                            lhsT=cID8[0:cn, :, 0:cn],
                            rhs=cMK[ci][:, s],
                            start=True,
                            stop=False,
                            perf_mode=PM.DoubleRow,
                        )
                for h in range(H):
                    nc.tensor.matmul(
                        d[0:cn, h % 2, 199 * (h // 2) : 199 * (h // 2) + N],
                        lhsT=cK2[64 * g : 64 * g + 64, h, co : co + cn],
                        rhs=fs[64 * g : 64 * g + 64, h // 2,
                               199 * (h % 2) : 199 * (h % 2) + N],
                        start=(not mask8
                               or os.environ.get("MASKOFF", "0") == "1"),
                        stop=True,
                        tile_position=(64 * g, 0),
                    )
                e = epool.tile([cn, 2, 2 * N], BF, tag=f"eT{ci}")
                nc.scalar.activation(e[:], d[0:cn, :, 0 : 2 * N], AF.Exp)
                return e

            go = b0 % OG
            if go == 0:
                sog = [
                    opool.tile([rn, OG, OUTC], BF, tag=f"sO{ci}", name=f"sOg{ci}")
                    for ci, (ro, rn) in enumerate(_CHUNKS)
                ]

            # item 0's dense through the dnu ring, item 1's through the oU
            # ring: the next pair's step1 then reuses the buffer freed by the
            # FIRST exp, so it overlaps the tail of this pair's exp drain.
            dp1 = po if os.environ.get("D1P", "0") == "1" else pd
            eB0 = [dense_chunk(0, 0, pd), dense_chunk(0, 1, pd)]
            eB1 = [dense_chunk(1, 0, dp1), dense_chunk(1, 1, dp1)]
            eTg = [eB0, eB1]

            oU2 = [po.tile([128, 2, 512], DT, tag="oU", name=f"oUF{ci}")
                   for ci in range(2)]
            # head-3 mask lives (as bf16) in the padding of the oU2 slots:
            # slot bytes [0:1040) hold the 260-col step5 output, [1040:1840)
            # hold the transposed e3 row-chunk for the same r-range.
            pRv = [oU2[ci][:].bitcast(BF) for ci in range(2)]

            def transposes(g, eT):
                # head-3 mask transposed into (r, c): PE-transpose of e3
                for rj, (ro, rn) in enumerate(_CHUNKS):
                    for ci, (co, cn) in enumerate(_CHUNKS):
                        nc.tensor.transpose(
                            pRv[rj][0:rn, g, 520 + co : 520 + co + cn],
                            in_=eT[ci][:, 1, N + ro : N + ro + rn],
                            identity=cID[0:cn, 0:cn],
                        )

            def step5(g, ci, eT):
                ro, rn = _CHUNKS[ci]
                for h in range(H):
                    for cc, (co, cn) in enumerate(_CHUNKS):
                        nc.tensor.matmul(
                            oU2[ci][0:rn, g, 65 * h : 65 * h + 65],
                            lhsT=eT[cc][
                                :, h % 2,
                                199 * (h // 2) + ro : 199 * (h // 2) + ro + rn,
                            ],
                            rhs=cXTall[0:cn, b0 + g, cc, 65 * h : 65 * h + 65],
                            start=(cc == 0),
                            stop=(cc == 1),
                        )

            transposes(0, eB0)
            step5(0, 0, eB0)
            step5(0, 1, eB0)
            # pipelined step1 for the NEXT pair sits here: PE-independent of
            # this pair's remaining exps, fills the wait for exp(b1).
            if b0 + 2 < bpc:
                if (b0 + 2) % BG == 0:
                    xg = load_xgroup(b0 + 2)
                fs_next = step1(xg, b0 + 2, q + 1)
            transposes(1, eB1)
            step5(1, 0, eB1)
            step5(1, 1, eB1)

            for ci, (ro, rn) in enumerate(_CHUNKS):
                oUF2 = oU2[ci]
                # 1/s for both items x 4 heads: s at col 64 of each 65-block
                rec = rpool.tile([rn, 2, H], DT, tag=f"rec{ci}")
                oUh = oUF2[0:rn, :, 0 : H * FA].rearrange(
                    "p g (h j) -> p g h j", j=FA
                )
                nc.vector.reciprocal(rec[:], oUh[:, :, :, 64])

                sO2 = sog[ci][:, go : go + 2]
                nc.vector.tensor_tensor(
                    sO2[:, :, 0 : H * U].rearrange("p g (h u) -> p g h u", u=U),
                    oUh[:, :, :, 0:U],
                    rec[:, :, :, None].to_broadcast((rn, 2, H, U)),
                    ALU.mult,
                )
                # mask3 normalize as per-item ACT scale-copies: these have no
                # fan-in until the pair's very end, so they execute in the
                # ACT gap at the next pair's start (exp waits on dense there).
                if os.environ.get("M3E", "vector") == "scalar":
                    for g in range(2):
                        nc.scalar.activation(
                            sO2[:, g, H * U : OUTC],
                            pRv[ci][0:rn, g, 520 : 520 + N],
                            AF.Copy,
                            scale=rec[:, g, 3:4],
                        )
                else:
                    nc.vector.tensor_tensor(
                        sO2[:, :, H * U : OUTC],
                        pRv[ci][0:rn, :, 520 : 520 + N],
                        rec[:, :, 3:4].to_broadcast((rn, 2, N)),
                        ALU.mult,
                    )

                if (go + 2 == OG or b0 + 2 >= bpc) and (
                    os.environ.get("SKIPO", "0") != "1"
                ):
                    # O is laid out (group, N, OG, OUTC) so each partition row
                    # writes one contiguous OG*OUTC*2-byte run; the host
                    # un-permutes the (group, OG) split afterwards.
                    nc.sync.dma_start(
                        O[b0 // OG, ro : ro + rn], sog[ci][:]
                    )


def build_nc(
    bpc: int = BPC, num_devices: int = NCORES, repeat: int = 1
) -> bass.Bass:
    nc = bacc.Bacc(
        "TRN2",
        target_bir_lowering=False,
        debug=False,
        num_devices=num_devices,
    )
    io = {
        "Xf": nc.dram_tensor("Xf", [bpc, N, F], BF, kind="ExternalInput").ap(),
        "XaT": nc.dram_tensor("XaT", [bpc, FA, N], BF, kind="ExternalInput").ap(),
        "AWp": nc.dram_tensor("AWp", [N, 2, 2 * N], BF, kind="ExternalInput").ap(),
        "K2": nc.dram_tensor("K2", [128, H, N], BF, kind="ExternalInput").ap(),
        "MK8": nc.dram_tensor("MK8", [N, 2, 2, 2 * N], F8, kind="ExternalInput").ap(),
        "ID8": nc.dram_tensor("ID8", [128, 2, 128], F8, kind="ExternalInput").ap(),
        "TA": nc.dram_tensor("TA", [FA, H, FA], BF, kind="ExternalInput").ap(),
        "ID": nc.dram_tensor("ID", [128, 128], BF, kind="ExternalInput").ap(),
        "O": nc.dram_tensor(
            "O", [bpc // OGRP, N, OGRP, OUTC], BF, kind="ExternalOutput"
        ).ap(),
    }
    with tile.TileContext(nc) as tc:
        if repeat == 1:
            _build_kernel_v3(nc, tc, io, bpc=bpc)
        else:
            # Timing-only variant: re-run the identical workload `repeat`
            # times in a hardware loop so per-dispatch tunnel latency can be
            # amortized out of the hardware-time measurement.
            import os as _os

            if _os.environ.get("STAGR", "1") == "1":
                with tc.For_i(0, repeat, 1, staggered_reset=True):
                    _build_kernel_v3(nc, tc, io, bpc=bpc)
            else:
                with tc.For_i(0, repeat, 1):
                    _build_kernel_v3(nc, tc, io, bpc=bpc)
    nc.compile()
    return nc


def _prep_weights(A, W, kernel, T, bias1, bias2):
    """Host-side constant prep (tiny tensors)."""
    A = np.asarray(A, np.float32)
    W = np.asarray(W, np.float32)
    kernel = np.asarray(kernel, np.float32)
    T = np.asarray(T, np.float32)
    bias1 = np.asarray(bias1, np.float32)
    bias2 = np.asarray(bias2, np.float32)

    AW = A[None, :, :] * W  # (H, n, m)
    # AWp[n, hp, k*199+m] = AW[2hp+k][n, m]
    AWp = np.ascontiguousarray(
        AW.reshape(2, 2, N, N).transpose(2, 0, 1, 3).reshape(N, 2, 2 * N)
    )

    Kf = kernel  # (H, F, N): [h, f, c]
    K1 = np.ascontiguousarray(Kf.transpose(1, 0, 2))  # [f, h, c]
    K2 = np.concatenate([K1, K1], axis=0)  # duplicate f-rows for PE rows 64-127

    # MK[c, h, m] = bias1[h, c] - 60 * (1 - A[m, c]): additive logit fixup
    # (adjacency mask + bias1); -60 and 0 are exactly representable in fp8e4.
    # Packed as [c, s, ktile, k*199 + m] with h = 2k + s; ktile 1 is zeros
    # (the second DoubleRow contraction tile contributes nothing).
    MK = bias1.T[:, :, None] - 60.0 * (1.0 - A.T[:, None, :])  # (c, h, m)
    MKs = MK.reshape(N, 2, 2, N).transpose(0, 2, 1, 3).reshape(N, 2, 2 * N)
    MK8 = np.zeros((N, 2, 2, 2 * N), np.float32)
    MK8[:, :, 0, :] = MKs

    # T_aug[h]: (65, 65): rows 0-63 = T[h], row 64 = [bias2[h], 1.0-at-col-64]
    TA = np.zeros((FA, H, FA), np.float32)
    TA[:F, :, :U] = T.transpose(1, 0, 2)
    TA[F, :, :U] = bias2
    TA[F, :, U] = 1.0

    ID8 = np.zeros((128, 2, 128), np.float32)
    ID8[:, 0, :] = np.eye(128, dtype=np.float32)

    import ml_dtypes

    bf = ml_dtypes.bfloat16
    f8 = ml_dtypes.float8_e4m3
    return dict(
        AWp=AWp.astype(bf), K2=K2.astype(bf), MK8=MK8.astype(f8),
        ID8=ID8.astype(f8), TA=TA.astype(bf), ID=np.eye(128, dtype=bf),
    )


_CACHED = {}


def _get_executable(repeat: int = 1):
    """Build the Bass module once and wrap it in a reusable sharded jax jit.

    Mirrors concourse.bass2jax.run_bass_via_pjrt's multi-core path, but caches
    the jitted callable so repeated kernel() calls skip re-lowering the BIR.
    """
    if repeat in _CACHED:
        return _CACHED[repeat]

    import jax
    from jax.sharding import Mesh, PartitionSpec
    from jax.experimental.shard_map import shard_map

    import concourse.mybir as _mybir
    from concourse import bass2jax

    bass2jax.install_neuronx_cc_hook()
    nc = build_nc(repeat=repeat)

    partition_name = (
        nc.partition_id_tensor.name if nc.partition_id_tensor else None
    )
    in_names, out_names, out_avals = [], [], []
    for alloc in nc.m.functions[0].allocations:
        if not isinstance(alloc, _mybir.MemoryLocationSet):
            continue
        name = alloc.memorylocations[0].name
        if alloc.kind == "ExternalInput":
            if name != partition_name:
                in_names.append(name)
        elif alloc.kind == "ExternalOutput":
            out_names.append(name)
            out_avals.append(
                jax.core.ShapedArray(
                    tuple(alloc.tensor_shape), _mybir.dt.np(alloc.dtype)
                )
            )
    n_params = len(in_names)
    n_outs = len(out_avals)
    all_in_names = list(in_names) + list(out_names)
    if partition_name is not None:
        all_in_names.append(partition_name)

    def _body(*args):
        operands = list(args)
        if partition_name is not None:
            operands.append(bass2jax.partition_id_tensor())
        outs = bass2jax._bass_exec_p.bind(
            *operands,
            out_avals=tuple(out_avals),
            in_names=tuple(all_in_names),
            out_names=tuple(out_names),
            lowering_input_output_aliases=(),
            sim_require_finite=True,
            sim_require_nnan=True,
            nc=nc,
        )
        return tuple(outs)

    devices = jax.devices()[:NCORES]
    mesh = Mesh(np.asarray(devices), ("core",))
    in_specs = (PartitionSpec("core"),) * (n_params + n_outs)
    out_specs = (PartitionSpec("core"),) * n_outs
    sharded = jax.jit(
        shard_map(
            _body, mesh=mesh, in_specs=in_specs, out_specs=out_specs,
            check_rep=False,
        ),
        donate_argnums=tuple(range(n_params, n_params + n_outs)),
        keep_unused=True,
    )
    _CACHED[repeat] = (sharded, in_names, out_names, out_avals, jax, mesh)
    return _CACHED[repeat]


def _stage_inputs(inputs):
    import ml_dtypes

    X = np.asarray(inputs["X"], np.float32)
    consts = _prep_weights(
        inputs["A"], inputs["W"], inputs["kernel"], inputs["T"],
        inputs["bias1"], inputs["bias2"],
    )
    bf = ml_dtypes.bfloat16
    Xb = X.astype(bf)
    XaT = np.concatenate(
        [X.transpose(0, 2, 1), np.ones((B, 1, N), np.float32)], axis=1
    ).astype(bf)
    per_core = {
        "Xf": np.ascontiguousarray(Xb),
        "XaT": np.ascontiguousarray(XaT),
    }
    for k, v in consts.items():
        per_core[k] = np.concatenate([v] * NCORES, axis=0)
    return per_core


def _run(staged):
    sharded, in_names, out_names, out_avals, jax, mesh = _get_executable()
    concat_in = [staged[nm] for nm in in_names]
    zeros = [
        np.zeros((NCORES * a.shape[0], *a.shape[1:]), a.dtype) for a in out_avals
    ]
    out_arrs = sharded(*concat_in, *zeros)
    return np.asarray(out_arrs[out_names.index("O")])


def kernel(**inputs) -> np.ndarray:
    staged = _stage_inputs(inputs)
    out = _run(staged)  # (NCORES*(BPC//OGRP), N, OGRP, OUTC) bf16
    out = out.astype(np.float32)
    out = out.reshape(NCORES, BPC // OGRP, N, OGRP, OUTC)
    out = out.transpose(0, 1, 3, 2, 4).reshape(B, N, OUTC)
    return np.ascontiguousarray(out)


# revision 25
# speedup vs baseline: 1.2399x; 1.2206x over previous
"""Trainium2 Bass kernel for DGCRNNCell (nn_DGCRNNCell_21792664060192).

Computes, for each batch item b and head h over graph with N=199 nodes:
  feat   = einsum('nf,nm->mf', X[b], A*W[h])          (via featT chain)
  dense  = feat @ kernel[h] + bias1[h]
  mask   = softmax(dense - NEG*(1-A), axis=-1)        (adjacency-masked softmax)
  node   = mask @ X[b]
  out_h  = node @ T[h] + bias2[h]
  output[b] = concat([out_0..out_3 (r, 256)], mask_3 (r, 199))   -> (199, 455)

Sharding: pure data-parallel over batch (512 -> 64 per core x 8 cores).

v3 dataflow (per core), built around item PAIRS and engine balance
(GPSIMD cannot touch PSUM on TRN2, so all PSUM-side elementwise work is
split between ACT and DVE with as few, as large instructions as possible):
  step1  featT for a pair (b0,b1): lhsT = [X[b0] | X[b1]] (cn, 128) so the
         pair's f-rows land on partitions 0-63 / 64-127; rhs = AW head-pair
         (cn, 2*199).  4 matmuls of free 398 per pair.
  fs     PSUM->SBUF bf16 copy of the pair's featT; alternates ACT/DVE.
  dense  per item: adjacency mask + bias1 written via an fp8 DoubleRow
         identity matmul (half cycles; -60/0 are exact in fp8e4), then 8
         bf16 matmuls accumulate kernel[h]^T @ featT; exp on ACT gives the
         masked e directly (one activation per c-chunk).
  XT     per item: X_aug @ T_aug, ONE matmul per c-chunk (free 260);
         col 64 of each head block = ones -> s; TA row 64 = bias2.
  step5  out = (e_h)^T @ XT_h accumulated over c-chunks into a 2-item PSUM
         tile; head-3 mask via PE transpose of e3 into a 2-item PSUM tile.
  stage  per item pair: ONE reciprocal, ONE normalize-multiply (512 free)
         and ONE mask3-multiply (398 free) on DVE; output staged bf16
         (host casts to fp32).
PSUM budget (8 banks): ring{fAB,d0,d1,XT} 2x2 banks; oUF2 2 banks;
pR2 2x1 banks.
"""

import os as _os

import numpy as np

import concourse.bass as bass
import concourse.mybir as mybir
import concourse.tile as tile
from concourse import bacc

if _os.environ.get("LDWOPT", "0") == "1":
    # The PE spends real time reloading stationary weights before every
    # matmul; walrus's ldweights-dedup optimization (its own default) is
    # pinned off by bass's production caller — turn it back on for this
    # kernel's compile.
    import concourse.bass_utils as _bu

    if not getattr(_bu, "_ldwopt_patched", False):
        _orig_run_command = _bu.run_command

        def _run_command_ldwopt(argv, **kwargs):
            argv = [
                "--enable-ldw-opt=true" if a == "--enable-ldw-opt=false" else a
                for a in argv
            ]
            return _orig_run_command(argv, **kwargs)

        _bu.run_command = _run_command_ldwopt
        _bu._ldwopt_patched = True

B, N, F, U, H = 512, 199, 64, 64, 4
NCORES = 8
BPC = B // NCORES  # 64 batch items per core
P0 = 128
P1 = N - P0  # 71
FA = F + 1  # X augmented with ones column (XaT row 64 = ones)
OUTC = H * U + N  # 455
DT = mybir.dt.float32
BF = mybir.dt.bfloat16
F8 = mybir.dt.float8e4
AF = mybir.ActivationFunctionType
ALU = mybir.AluOpType
PM = mybir.MatmulPerfMode

_CHUNKS = ((0, P0), (P0, P1))  # (offset, size) along the N(=c or r) axis
OGRP = 4  # output DMA item-group (must match the OG default below)


def _build_kernel_v3(nc: bass.Bass, tc: "tile.TileContext", io: dict, bpc: int = BPC):
    import os
    from contextlib import ExitStack

    Xf, XaT, AWp, K2, MK8, ID8, TA, ID, O = (
        io["Xf"], io["XaT"], io["AWp"], io["K2"], io["MK8"], io["ID8"],
        io["TA"], io["ID"], io["O"],
    )

    def _b(name, default):
        return int(os.environ.get(name, str(default)))

    fse = os.environ.get("FSE", "alt")     # fs copy engine: alt|scalar|vector
    mask8 = os.environ.get("MASK8", "1") == "1"  # fp8 DoubleRow mask write

    with ExitStack() as ctx:
        cpool = ctx.enter_context(tc.tile_pool(name="consts", bufs=1))
        xpool = ctx.enter_context(tc.tile_pool(name="xf", bufs=_b("XB", 2)))
        fspool = ctx.enter_context(tc.tile_pool(name="fs", bufs=_b("FSB", 2)))
        epool = ctx.enter_context(tc.tile_pool(name="expT", bufs=_b("EB", 3)))
        rpool = ctx.enter_context(tc.tile_pool(name="rec", bufs=_b("RB", 4)))
        opool = ctx.enter_context(tc.tile_pool(name="sO", bufs=_b("OB", 2)))

        # ---- constants into SBUF (once) ----
        cAW = []
        cMK = []
        for ci, (co, cn) in enumerate(_CHUNKS):
            t = cpool.tile([cn, 2, 2 * N], BF, name=f"cAW{ci}")
            nc.sync.dma_start(t[:], AWp[co : co + cn])
            cAW.append(t)
            if mask8:
                t = cpool.tile([cn, 2, 2, 2 * N], F8, name=f"cMK{ci}")
                nc.sync.dma_start(t[:], MK8[co : co + cn])
                cMK.append(t)
        cK2 = cpool.tile([128, H, N], BF, name="cK2")
        cTA = cpool.tile([FA, H, FA], BF, name="cTA")
        cID = cpool.tile([128, 128], BF, name="cID")
        nc.sync.dma_start(cK2[:], K2[:])
        nc.sync.dma_start(cTA[:], TA[:])
        nc.sync.dma_start(cID[:], ID[:])
        if mask8:
            cID8 = cpool.tile([128, 2, 128], F8, name="cID8")
            nc.sync.dma_start(cID8[:], ID8[:])

        BG = min(_b("BG", 8), bpc)   # input DMA batching
        OG = OGRP  # output DMA batching (matches the DRAM layout)

        # ---- prologue: XT = Xa_aug @ TA_aug for ALL items, kept in SBUF ----
        # cXTall[c, b, cc, 65h + j]: cols 0-63 of each head block = XT data,
        # col 64 = ones (the softmax-denominator column), written once.
        cXTall = cpool.tile([128, bpc, 2, H * FA], BF, name="cXTall")
        nc.vector.memset(
            cXTall[:].rearrange("p b c (h j) -> p b c h j", j=FA)[:, :, :, :, 64],
            1.0,
        )
        PG = 4  # items per prologue PSUM tile
        with tc.tile_pool(name="pxt", bufs=_b("XTB", 2), space="PSUM") as pxt:
            for b0 in range(0, bpc, PG):
                if b0 % BG == 0:
                    ng = min(BG, bpc - b0)
                    xtg = xpool.tile([FA, BG * N], BF, tag="xat")
                    nc.sync.dma_start(
                        xtg[:, 0 : ng * N].rearrange("j (g n) -> j g n", n=N),
                        XaT[b0 : b0 + ng].rearrange("g j n -> j g n"),
                    )
                gi = b0 % BG
                XTp = pxt.tile([128, PG, 2, 256], DT, tag="xtp")
                for g in range(PG):
                    xt = xtg[:, (gi + g) * N : (gi + g + 1) * N]
                    for ci, (co, cn) in enumerate(_CHUNKS):
                        nc.tensor.matmul(
                            XTp[0:cn, g, ci, :],
                            lhsT=xt[:, co : co + cn],
                            rhs=cTA[:, :, 0:U],
                            start=True,
                            stop=True,
                        )
                dst = cXTall[:, b0 : b0 + PG].rearrange(
                    "p b c (h j) -> p b c h j", j=FA
                )[:, :, :, :, 0:U]
                if (b0 // PG) % 2 == 0:
                    nc.scalar.copy(
                        dst, XTp[:].rearrange("p b c (h u) -> p b c h u", u=U)
                    )
                else:
                    nc.vector.tensor_copy(
                        dst, XTp[:].rearrange("p b c (h u) -> p b c h u", u=U)
                    )

        pd = ctx.enter_context(
            tc.tile_pool(name="pdnu", bufs=_b("DTB", 2), space="PSUM")
        )
        po = ctx.enter_context(
            tc.tile_pool(name="poU", bufs=_b("POB", 2), space="PSUM")
        )

        def load_xgroup(b0):
            ng = min(BG, bpc - b0)
            src = Xf[b0 : b0 + ng].rearrange("g n f -> n g f")
            xg = []
            for ci, (co, cn) in enumerate(_CHUNKS):
                t = xpool.tile([cn, BG, F], BF, tag=f"xf{ci}")
                nc.sync.dma_start(t[:, 0:ng, :], src[co : co + cn])
                xg.append(t)
            return xg

        def step1(xg, b0, q):
            # step1 for a pair: out partitions = [b0 f | b1 f].  Allocated
            # from the dnu ring; emitted one pair AHEAD so the copy+dense of
            # pair q+1 overlap the exp drain of pair q.
            gi = b0 % BG
            fAB = pd.tile([128, 2, 512], DT, tag="dnu", name="fAB")
            for hp in range(2):
                for ci, (co, cn) in enumerate(_CHUNKS):
                    nc.tensor.matmul(
                        fAB[:, hp, 0 : 2 * N],
                        lhsT=xg[ci][:, gi : gi + 2, :],
                        rhs=cAW[ci][:, hp, :],
                        start=(ci == 0),
                        stop=(ci == 1),
                    )
            fs = fspool.tile([128, 2, 2 * N], BF, tag="fs")
            if fse == "split":
                nc.scalar.copy(fs[:, 0], fAB[:, 0, 0 : 2 * N])
                nc.vector.tensor_copy(fs[:, 1], fAB[:, 1, 0 : 2 * N])
            elif fse == "scalar" or (fse == "alt" and q % 2 == 0):
                nc.scalar.copy(fs[:], fAB[:, :, 0 : 2 * N])
            else:
                nc.vector.tensor_copy(fs[:], fAB[:, :, 0 : 2 * N])
            return fs

        xg = load_xgroup(0)
        fs_next = step1(xg, 0, 0)
        sog = [None, None]
        for q in range(bpc // 2):
            b0 = 2 * q
            fs = fs_next

            def dense_chunk(g, ci, pool):
                # dense chunk + exp; head h -> slot s=h%2, block k=h//2
                co, cn = _CHUNKS[ci]
                d = pool.tile([128, 2, 512], DT,
                              tag="dnu" if pool is pd else "oU",
                              name=f"dT{g}{ci}")
                if mask8 and os.environ.get("MASKOFF", "0") != "1":
                    for s in range(2):
                        nc.tensor.matmul(
                            d[0:cn, s, 0 : 2 * N],
                            lhsT=cID8[0:cn, :, 0:cn],
                            rhs=cMK[ci][:, s],
                            start=True,
                            stop=False,
                            perf_mode=PM.DoubleRow,
                        )
                for h in range(H):
                    nc.tensor.matmul(
                        d[0:cn, h % 2, 199 * (h // 2) : 199 * (h // 2) + N],
                        lhsT=cK2[64 * g : 64 * g + 64, h, co : co + cn],
                        rhs=fs[64 * g : 64 * g + 64, h // 2,
                               199 * (h % 2) : 199 * (h % 2) + N],
                        start=(not mask8
                               or os.environ.get("MASKOFF", "0") == "1"),
                        stop=True,
                        tile_position=(64 * g, 0),
                    )
                e = epool.tile([cn, 2, 2 * N], BF, tag=f"eT{ci}")
                nc.scalar.activation(e[:], d[0:cn, :, 0 : 2 * N], AF.Exp)
                return e

            go = b0 % OG
            if go == 0:
                sog = [
                    opool.tile([rn, OG, OUTC], BF, tag=f"sO{ci}", name=f"sOg{ci}")
                    for ci, (ro, rn) in enumerate(_CHUNKS)
                ]

            # item 0's dense through the dnu ring, item 1's through the oU
            # ring: the next pair's step1 then reuses the buffer freed by the
            # FIRST exp, so it overlaps the tail of this pair's exp drain.
            dp1 = po if os.environ.get("D1P", "0") == "1" else pd
            eB0 = [dense_chunk(0, 0, pd), dense_chunk(0, 1, pd)]
            eB1 = [dense_chunk(1, 0, dp1), dense_chunk(1, 1, dp1)]
            eTg = [eB0, eB1]

            oU2 = [po.tile([128, 2, 512], DT, tag="oU", name=f"oUF{ci}")
                   for ci in range(2)]
            # head-3 mask lives (as bf16) in the padding of the oU2 slots:
            # slot bytes [0:1040) hold the 260-col step5 output, [1040:1840)
            # hold the transposed e3 row-chunk for the same r-range.
            pRv = [oU2[ci][:].bitcast(BF) for ci in range(2)]

            def transposes(g, eT):
                # head-3 mask transposed into (r, c): PE-transpose of e3
                for rj, (ro, rn) in enumerate(_CHUNKS):
                    for ci, (co, cn) in enumerate(_CHUNKS):
                        nc.tensor.transpose(
                            pRv[rj][0:rn, g, 520 + co : 520 + co + cn],
                            in_=eT[ci][:, 1, N + ro : N + ro + rn],
                            identity=cID[0:cn, 0:cn],
                        )

            def step5(g, ci, eT):
                ro, rn = _CHUNKS[ci]
                for h in range(H):
                    for cc, (co, cn) in enumerate(_CHUNKS):
                        nc.tensor.matmul(
                            oU2[ci][0:rn, g, 65 * h : 65 * h + 65],
                            lhsT=eT[cc][
                                :, h % 2,
                                199 * (h // 2) + ro : 199 * (h // 2) + ro + rn,
                            ],
                            rhs=cXTall[0:cn, b0 + g, cc, 65 * h : 65 * h + 65],
                            start=(cc == 0),
                            stop=(cc == 1),
                        )

            transposes(0, eB0)
            step5(0, 0, eB0)
            step5(0, 1, eB0)
            # pipelined step1 for the NEXT pair sits here: PE-independent of
            # this pair's remaining exps, fills the wait for exp(b1).
            if b0 + 2 < bpc:
                if (b0 + 2) % BG == 0:
                    xg = load_xgroup(b0 + 2)
                fs_next = step1(xg, b0 + 2, q + 1)
            transposes(1, eB1)
            step5(1, 0, eB1)
            step5(1, 1, eB1)

            for ci, (ro, rn) in enumerate(_CHUNKS):
                oUF2 = oU2[ci]
                # 1/s for both items x 4 heads: s at col 64 of each 65-block
                rec = rpool.tile([rn, 2, H], DT, tag=f"rec{ci}")
                oUh = oUF2[0:rn, :, 0 : H * FA].rearrange(
                    "p g (h j) -> p g h j", j=FA
                )
                nc.vector.reciprocal(rec[:], oUh[:, :, :, 64])

                sO2 = sog[ci][:, go : go + 2]
                nc.vector.tensor_tensor(
                    sO2[:, :, 0 : H * U].rearrange("p g (h u) -> p g h u", u=U),
                    oUh[:, :, :, 0:U],
                    rec[:, :, :, None].to_broadcast((rn, 2, H, U)),
                    ALU.mult,
                )
                # mask3 normalize as per-item ACT scale-copies: these have no
                # fan-in until the pair's very end, so they execute in the
                # ACT gap at the next pair's start (exp waits on dense there).
                if os.environ.get("M3E", "vector") == "scalar":
                    for g in range(2):
                        nc.scalar.activation(
                            sO2[:, g, H * U : OUTC],
                            pRv[ci][0:rn, g, 520 : 520 + N],
                            AF.Copy,
                            scale=rec[:, g, 3:4],
                        )
                else:
                    nc.vector.tensor_tensor(
                        sO2[:, :, H * U : OUTC],
                        pRv[ci][0:rn, :, 520 : 520 + N],
                        rec[:, :, 3:4].to_broadcast((rn, 2, N)),
                        ALU.mult,
                    )

                if (go + 2 == OG or b0 + 2 >= bpc) and (
                    os.environ.get("SKIPO", "0") != "1"
                ):
                    # O is laid out (group, N, OG, OUTC) so each partition row
                    # writes one contiguous OG*OUTC*2-byte run; the host
                    # un-permutes the (group, OG) split afterwards.
                    oq = nc.sync
                    if os.environ.get("OQ2", "1") == "1" and ci == 1:
                        oq = nc.gpsimd
                    oq.dma_start(O[b0 // OG, ro : ro + rn], sog[ci][:])


def build_nc(
    bpc: int = BPC, num_devices: int = NCORES, repeat: int = 1
) -> bass.Bass:
    nc = bacc.Bacc(
        "TRN2",
        target_bir_lowering=False,
        debug=False,
        num_devices=num_devices,
    )
    io = {
        "Xf": nc.dram_tensor("Xf", [bpc, N, F], BF, kind="ExternalInput").ap(),
        "XaT": nc.dram_tensor("XaT", [bpc, FA, N], BF, kind="ExternalInput").ap(),
        "AWp": nc.dram_tensor("AWp", [N, 2, 2 * N], BF, kind="ExternalInput").ap(),
        "K2": nc.dram_tensor("K2", [128, H, N], BF, kind="ExternalInput").ap(),
        "MK8": nc.dram_tensor("MK8", [N, 2, 2, 2 * N], F8, kind="ExternalInput").ap(),
        "ID8": nc.dram_tensor("ID8", [128, 2, 128], F8, kind="ExternalInput").ap(),
        "TA": nc.dram_tensor("TA", [FA, H, FA], BF, kind="ExternalInput").ap(),
        "ID": nc.dram_tensor("ID", [128, 128], BF, kind="ExternalInput").ap(),
        "O": nc.dram_tensor(
            "O", [bpc // OGRP, N, OGRP, OUTC], BF, kind="ExternalOutput"
        ).ap(),
    }
    with tile.TileContext(nc) as tc:
        if repeat == 1:
            _build_kernel_v3(nc, tc, io, bpc=bpc)
        else:
            # Timing-only variant: re-run the identical workload `repeat`
            # times in a hardware loop so per-dispatch tunnel latency can be
            # amortized out of the hardware-time measurement.
            import os as _os

            if _os.environ.get("STAGR", "1") == "1":
                with tc.For_i(0, repeat, 1, staggered_reset=True):
                    _build_kernel_v3(nc, tc, io, bpc=bpc)
            else:
                with tc.For_i(0, repeat, 1):
                    _build_kernel_v3(nc, tc, io, bpc=bpc)
    nc.compile()
    return nc


def _prep_weights(A, W, kernel, T, bias1, bias2):
    """Host-side constant prep (tiny tensors)."""
    A = np.asarray(A, np.float32)
    W = np.asarray(W, np.float32)
    kernel = np.asarray(kernel, np.float32)
    T = np.asarray(T, np.float32)
    bias1 = np.asarray(bias1, np.float32)
    bias2 = np.asarray(bias2, np.float32)

    AW = A[None, :, :] * W  # (H, n, m)
    # AWp[n, hp, k*199+m] = AW[2hp+k][n, m]
    AWp = np.ascontiguousarray(
        AW.reshape(2, 2, N, N).transpose(2, 0, 1, 3).reshape(N, 2, 2 * N)
    )

    Kf = kernel  # (H, F, N): [h, f, c]
    K1 = np.ascontiguousarray(Kf.transpose(1, 0, 2))  # [f, h, c]
    K2 = np.concatenate([K1, K1], axis=0)  # duplicate f-rows for PE rows 64-127

    # MK[c, h, m] = bias1[h, c] - 60 * (1 - A[m, c]): additive logit fixup
    # (adjacency mask + bias1); -60 and 0 are exactly representable in fp8e4.
    # Packed as [c, s, ktile, k*199 + m] with h = 2k + s; ktile 1 is zeros
    # (the second DoubleRow contraction tile contributes nothing).
    MK = bias1.T[:, :, None] - 60.0 * (1.0 - A.T[:, None, :])  # (c, h, m)
    MKs = MK.reshape(N, 2, 2, N).transpose(0, 2, 1, 3).reshape(N, 2, 2 * N)
    MK8 = np.zeros((N, 2, 2, 2 * N), np.float32)
    MK8[:, :, 0, :] = MKs

    # T_aug[h]: (65, 65): rows 0-63 = T[h], row 64 = [bias2[h], 1.0-at-col-64]
    TA = np.zeros((FA, H, FA), np.float32)
    TA[:F, :, :U] = T.transpose(1, 0, 2)
    TA[F, :, :U] = bias2
    TA[F, :, U] = 1.0

    ID8 = np.zeros((128, 2, 128), np.float32)
    ID8[:, 0, :] = np.eye(128, dtype=np.float32)

    import ml_dtypes

    bf = ml_dtypes.bfloat16
    f8 = ml_dtypes.float8_e4m3
    return dict(
        AWp=AWp.astype(bf), K2=K2.astype(bf), MK8=MK8.astype(f8),
        ID8=ID8.astype(f8), TA=TA.astype(bf), ID=np.eye(128, dtype=bf),
    )


_CACHED = {}


def _get_executable(repeat: int = 1):
    """Build the Bass module once and wrap it in a reusable sharded jax jit.

    Mirrors concourse.bass2jax.run_bass_via_pjrt's multi-core path, but caches
    the jitted callable so repeated kernel() calls skip re-lowering the BIR.
    """
    if repeat in _CACHED:
        return _CACHED[repeat]

    import jax
    from jax.sharding import Mesh, PartitionSpec
    from jax.experimental.shard_map import shard_map

    import concourse.mybir as _mybir
    from concourse import bass2jax

    bass2jax.install_neuronx_cc_hook()
    nc = build_nc(repeat=repeat)

    partition_name = (
        nc.partition_id_tensor.name if nc.partition_id_tensor else None
    )
    in_names, out_names, out_avals = [], [], []
    for alloc in nc.m.functions[0].allocations:
        if not isinstance(alloc, _mybir.MemoryLocationSet):
            continue
        name = alloc.memorylocations[0].name
        if alloc.kind == "ExternalInput":
            if name != partition_name:
                in_names.append(name)
        elif alloc.kind == "ExternalOutput":
            out_names.append(name)
            out_avals.append(
                jax.core.ShapedArray(
                    tuple(alloc.tensor_shape), _mybir.dt.np(alloc.dtype)
                )
            )
    n_params = len(in_names)
    n_outs = len(out_avals)
    all_in_names = list(in_names) + list(out_names)
    if partition_name is not None:
        all_in_names.append(partition_name)

    def _body(*args):
        operands = list(args)
        if partition_name is not None:
            operands.append(bass2jax.partition_id_tensor())
        outs = bass2jax._bass_exec_p.bind(
            *operands,
            out_avals=tuple(out_avals),
            in_names=tuple(all_in_names),
            out_names=tuple(out_names),
            lowering_input_output_aliases=(),
            sim_require_finite=True,
            sim_require_nnan=True,
            nc=nc,
        )
        return tuple(outs)

    devices = jax.devices()[:NCORES]
    mesh = Mesh(np.asarray(devices), ("core",))
    in_specs = (PartitionSpec("core"),) * (n_params + n_outs)
    out_specs = (PartitionSpec("core"),) * n_outs
    sharded = jax.jit(
        shard_map(
            _body, mesh=mesh, in_specs=in_specs, out_specs=out_specs,
            check_rep=False,
        ),
        donate_argnums=tuple(range(n_params, n_params + n_outs)),
        keep_unused=True,
    )
    _CACHED[repeat] = (sharded, in_names, out_names, out_avals, jax, mesh)
    return _CACHED[repeat]


def _stage_inputs(inputs):
    import ml_dtypes

    X = np.asarray(inputs["X"], np.float32)
    consts = _prep_weights(
        inputs["A"], inputs["W"], inputs["kernel"], inputs["T"],
        inputs["bias1"], inputs["bias2"],
    )
    bf = ml_dtypes.bfloat16
    Xb = X.astype(bf)
    XaT = np.concatenate(
        [X.transpose(0, 2, 1), np.ones((B, 1, N), np.float32)], axis=1
    ).astype(bf)
    per_core = {
        "Xf": np.ascontiguousarray(Xb),
        "XaT": np.ascontiguousarray(XaT),
    }
    for k, v in consts.items():
        per_core[k] = np.concatenate([v] * NCORES, axis=0)
    return per_core


def _run(staged):
    sharded, in_names, out_names, out_avals, jax, mesh = _get_executable()
    concat_in = [staged[nm] for nm in in_names]
    zeros = [
        np.zeros((NCORES * a.shape[0], *a.shape[1:]), a.dtype) for a in out_avals
    ]
    out_arrs = sharded(*concat_in, *zeros)
    return np.asarray(out_arrs[out_names.index("O")])


def kernel(**inputs) -> np.ndarray:
    staged = _stage_inputs(inputs)
    out = _run(staged)  # (NCORES*(BPC//OGRP), N, OGRP, OUTC) bf16
    out = out.astype(np.float32)
    out = out.reshape(NCORES, BPC // OGRP, N, OGRP, OUTC)
    out = out.transpose(0, 1, 3, 2, 4).reshape(B, N, OUTC)
    return np.ascontiguousarray(out)


# revision 26
# speedup vs baseline: 1.3130x; 1.0590x over previous
"""Trainium2 Bass kernel for DGCRNNCell (nn_DGCRNNCell_21792664060192).

Computes, for each batch item b and head h over graph with N=199 nodes:
  feat   = einsum('nf,nm->mf', X[b], A*W[h])          (via featT chain)
  dense  = feat @ kernel[h] + bias1[h]
  mask   = softmax(dense - NEG*(1-A), axis=-1)        (adjacency-masked softmax)
  node   = mask @ X[b]
  out_h  = node @ T[h] + bias2[h]
  output[b] = concat([out_0..out_3 (r, 256)], mask_3 (r, 199))   -> (199, 455)

Sharding: pure data-parallel over batch (512 -> 64 per core x 8 cores).

v3 dataflow (per core), built around item PAIRS and engine balance
(GPSIMD cannot touch PSUM on TRN2, so all PSUM-side elementwise work is
split between ACT and DVE with as few, as large instructions as possible):
  step1  featT for a pair (b0,b1): lhsT = [X[b0] | X[b1]] (cn, 128) so the
         pair's f-rows land on partitions 0-63 / 64-127; rhs = AW head-pair
         (cn, 2*199).  4 matmuls of free 398 per pair.
  fs     PSUM->SBUF bf16 copy of the pair's featT; alternates ACT/DVE.
  dense  per item: adjacency mask + bias1 written via an fp8 DoubleRow
         identity matmul (half cycles; -60/0 are exact in fp8e4), then 8
         bf16 matmuls accumulate kernel[h]^T @ featT; exp on ACT gives the
         masked e directly (one activation per c-chunk).
  XT     per item: X_aug @ T_aug, ONE matmul per c-chunk (free 260);
         col 64 of each head block = ones -> s; TA row 64 = bias2.
  step5  out = (e_h)^T @ XT_h accumulated over c-chunks into a 2-item PSUM
         tile; head-3 mask via PE transpose of e3 into a 2-item PSUM tile.
  stage  per item pair: ONE reciprocal, ONE normalize-multiply (512 free)
         and ONE mask3-multiply (398 free) on DVE; output staged bf16
         (host casts to fp32).
PSUM budget (8 banks): ring{fAB,d0,d1,XT} 2x2 banks; oUF2 2 banks;
pR2 2x1 banks.
"""

import os as _os

import numpy as np

import concourse.bass as bass
import concourse.mybir as mybir
import concourse.tile as tile
from concourse import bacc

if _os.environ.get("LDWOPT", "0") == "1":
    # The PE spends real time reloading stationary weights before every
    # matmul; walrus's ldweights-dedup optimization (its own default) is
    # pinned off by bass's production caller — turn it back on for this
    # kernel's compile.
    import concourse.bass_utils as _bu

    if not getattr(_bu, "_ldwopt_patched", False):
        _orig_run_command = _bu.run_command

        def _run_command_ldwopt(argv, **kwargs):
            argv = [
                "--enable-ldw-opt=true" if a == "--enable-ldw-opt=false" else a
                for a in argv
            ]
            return _orig_run_command(argv, **kwargs)

        _bu.run_command = _run_command_ldwopt
        _bu._ldwopt_patched = True

B, N, F, U, H = 512, 199, 64, 64, 4
NCORES = 8
BPC = B // NCORES  # 64 batch items per core
P0 = 128
P1 = N - P0  # 71
FA = F + 1  # X augmented with ones column (XaT row 64 = ones)
OUTC = H * U + N  # 455
DT = mybir.dt.float32
BF = mybir.dt.bfloat16
F8 = mybir.dt.float8e4
AF = mybir.ActivationFunctionType
ALU = mybir.AluOpType
PM = mybir.MatmulPerfMode

_CHUNKS = ((0, P0), (P0, P1))  # (offset, size) along the N(=c or r) axis
OGRP = 4  # output DMA item-group (must match the OG default below)


def _build_kernel_v3(nc: bass.Bass, tc: "tile.TileContext", io: dict, bpc: int = BPC):
    import os
    from contextlib import ExitStack

    Xf, XaT, AWp, K2, MK8, ID8, TA, ID, O = (
        io["Xf"], io["XaT"], io["AWp"], io["K2"], io["MK8"], io["ID8"],
        io["TA"], io["ID"], io["O"],
    )

    def _b(name, default):
        return int(os.environ.get(name, str(default)))

    fse = os.environ.get("FSE", "alt")     # fs copy engine: alt|scalar|vector
    mask8 = os.environ.get("MASK8", "1") == "1"  # fp8 DoubleRow mask write

    with ExitStack() as ctx:
        cpool = ctx.enter_context(tc.tile_pool(name="consts", bufs=1))
        xpool = ctx.enter_context(tc.tile_pool(name="xf", bufs=_b("XB", 2)))
        fspool = ctx.enter_context(tc.tile_pool(name="fs", bufs=_b("FSB", 2)))
        epool = ctx.enter_context(tc.tile_pool(name="expT", bufs=_b("EB", 3)))
        rpool = ctx.enter_context(tc.tile_pool(name="rec", bufs=_b("RB", 4)))
        opool = ctx.enter_context(tc.tile_pool(name="sO", bufs=_b("OB", 2)))

        # ---- constants into SBUF (once) ----
        cAW = []
        cMK = []
        for ci, (co, cn) in enumerate(_CHUNKS):
            t = cpool.tile([cn, 2, 2 * N], BF, name=f"cAW{ci}")
            nc.sync.dma_start(t[:], AWp[co : co + cn])
            cAW.append(t)
            if mask8:
                t = cpool.tile([cn, 2, 2, 2 * N], F8, name=f"cMK{ci}")
                nc.sync.dma_start(t[:], MK8[co : co + cn])
                cMK.append(t)
        cK2 = cpool.tile([128, H, N], BF, name="cK2")
        cTA = cpool.tile([FA, H, FA], BF, name="cTA")
        cID = cpool.tile([128, 128], BF, name="cID")
        nc.sync.dma_start(cK2[:], K2[:])
        nc.sync.dma_start(cTA[:], TA[:])
        nc.sync.dma_start(cID[:], ID[:])
        if mask8:
            cID8 = cpool.tile([128, 2, 128], F8, name="cID8")
            nc.sync.dma_start(cID8[:], ID8[:])

        BG = min(_b("BG", 8), bpc)   # input DMA batching
        OG = OGRP  # output DMA batching (matches the DRAM layout)

        # ---- prologue: XT = Xa_aug @ TA_aug for ALL items, kept in SBUF ----
        # cXTall[c, b, cc, 65h + j]: cols 0-63 of each head block = XT data,
        # col 64 = ones (the softmax-denominator column), written once.
        cXTall = cpool.tile([128, bpc, 2, H * FA], BF, name="cXTall")
        nc.vector.memset(
            cXTall[:].rearrange("p b c (h j) -> p b c h j", j=FA)[:, :, :, :, 64],
            1.0,
        )
        PG = 4  # items per prologue PSUM tile
        with tc.tile_pool(name="pxt", bufs=_b("XTB", 2), space="PSUM") as pxt:
            for b0 in range(0, bpc, PG):
                if b0 % BG == 0:
                    ng = min(BG, bpc - b0)
                    xtg = xpool.tile([FA, BG * N], BF, tag="xat")
                    nc.sync.dma_start(
                        xtg[:, 0 : ng * N].rearrange("j (g n) -> j g n", n=N),
                        XaT[b0 : b0 + ng].rearrange("g j n -> j g n"),
                    )
                gi = b0 % BG
                XTp = pxt.tile([128, PG, 2, 256], DT, tag="xtp")
                for g in range(PG):
                    xt = xtg[:, (gi + g) * N : (gi + g + 1) * N]
                    for ci, (co, cn) in enumerate(_CHUNKS):
                        nc.tensor.matmul(
                            XTp[0:cn, g, ci, :],
                            lhsT=xt[:, co : co + cn],
                            rhs=cTA[:, :, 0:U],
                            start=True,
                            stop=True,
                        )
                dst = cXTall[:, b0 : b0 + PG].rearrange(
                    "p b c (h j) -> p b c h j", j=FA
                )[:, :, :, :, 0:U]
                if (b0 // PG) % 2 == 0:
                    nc.scalar.copy(
                        dst, XTp[:].rearrange("p b c (h u) -> p b c h u", u=U)
                    )
                else:
                    nc.vector.tensor_copy(
                        dst, XTp[:].rearrange("p b c (h u) -> p b c h u", u=U)
                    )

        pd = ctx.enter_context(
            tc.tile_pool(name="pdnu", bufs=_b("DTB", 2), space="PSUM")
        )
        po = ctx.enter_context(
            tc.tile_pool(name="poU", bufs=_b("POB", 2), space="PSUM")
        )

        def load_xgroup(b0):
            ng = min(BG, bpc - b0)
            src = Xf[b0 : b0 + ng].rearrange("g n f -> n g f")
            xg = []
            for ci, (co, cn) in enumerate(_CHUNKS):
                t = xpool.tile([cn, BG, F], BF, tag=f"xf{ci}")
                nc.sync.dma_start(t[:, 0:ng, :], src[co : co + cn])
                xg.append(t)
            return xg

        def step1(xg, b0, q):
            # step1 for a pair: out partitions = [b0 f | b1 f].  Allocated
            # from the dnu ring; emitted one pair AHEAD so the copy+dense of
            # pair q+1 overlap the exp drain of pair q.
            gi = b0 % BG
            fAB = pd.tile([128, 2, 512], DT, tag="dnu", name="fAB")
            for hp in range(2):
                for ci, (co, cn) in enumerate(_CHUNKS):
                    nc.tensor.matmul(
                        fAB[:, hp, 0 : 2 * N],
                        lhsT=xg[ci][:, gi : gi + 2, :],
                        rhs=cAW[ci][:, hp, :],
                        start=(ci == 0),
                        stop=(ci == 1),
                    )
            fs = fspool.tile([128, 2, 2 * N], BF, tag="fs")
            if fse == "split":
                nc.scalar.copy(fs[:, 0], fAB[:, 0, 0 : 2 * N])
                nc.vector.tensor_copy(fs[:, 1], fAB[:, 1, 0 : 2 * N])
            elif fse == "scalar" or (fse == "alt" and q % 2 == 0):
                nc.scalar.copy(fs[:], fAB[:, :, 0 : 2 * N])
            else:
                nc.vector.tensor_copy(fs[:], fAB[:, :, 0 : 2 * N])
            return fs

        xg = load_xgroup(0)
        fs_next = step1(xg, 0, 0)
        sog = [None, None]
        for q in range(bpc // 2):
            b0 = 2 * q
            fs = fs_next

            def dense_chunk(g, ci, pool):
                # dense chunk + exp; head h -> slot s=h%2, block k=h//2
                co, cn = _CHUNKS[ci]
                d = pool.tile([128, 2, 512], DT,
                              tag="dnu" if pool is pd else "oU",
                              name=f"dT{g}{ci}")
                if mask8 and os.environ.get("MASKOFF", "0") != "1":
                    for s in range(2):
                        nc.tensor.matmul(
                            d[0:cn, s, 0 : 2 * N],
                            lhsT=cID8[0:cn, :, 0:cn],
                            rhs=cMK[ci][:, s],
                            start=True,
                            stop=False,
                            perf_mode=PM.DoubleRow,
                        )
                for h in range(H):
                    nc.tensor.matmul(
                        d[0:cn, h % 2, 199 * (h // 2) : 199 * (h // 2) + N],
                        lhsT=cK2[64 * g : 64 * g + 64, h, co : co + cn],
                        rhs=fs[64 * g : 64 * g + 64, h // 2,
                               199 * (h % 2) : 199 * (h % 2) + N],
                        start=(not mask8
                               or os.environ.get("MASKOFF", "0") == "1"),
                        stop=True,
                        tile_position=(64 * g, 0),
                    )
                e = epool.tile([cn, 2, 2 * N], BF, tag=f"eT{ci}")
                nc.scalar.activation(e[:], d[0:cn, :, 0 : 2 * N], AF.Exp)
                return e

            go = b0 % OG
            if go == 0:
                sog = [
                    opool.tile([rn, OG, OUTC], BF, tag=f"sO{ci}", name=f"sOg{ci}")
                    for ci, (ro, rn) in enumerate(_CHUNKS)
                ]

            # item 0's dense through the dnu ring, item 1's through the oU
            # ring: the next pair's step1 then reuses the buffer freed by the
            # FIRST exp, so it overlaps the tail of this pair's exp drain.
            dp1 = po if os.environ.get("D1P", "0") == "1" else pd
            eB0 = [dense_chunk(0, 0, pd), dense_chunk(0, 1, pd)]
            eB1 = [dense_chunk(1, 0, dp1), dense_chunk(1, 1, dp1)]
            eTg = [eB0, eB1]

            oU2 = [po.tile([128, 2, 512], DT, tag="oU", name=f"oUF{ci}")
                   for ci in range(2)]
            # head-3 mask lives (as bf16) in the padding of the oU2 slots:
            # slot bytes [0:1040) hold the 260-col step5 output, [1040:1840)
            # hold the transposed e3 row-chunk for the same r-range.
            pRv = [oU2[ci][:].bitcast(BF) for ci in range(2)]

            def transposes(g, eT):
                # head-3 mask transposed into (r, c): PE-transpose of e3
                for rj, (ro, rn) in enumerate(_CHUNKS):
                    for ci, (co, cn) in enumerate(_CHUNKS):
                        nc.tensor.transpose(
                            pRv[rj][0:rn, g, 520 + co : 520 + co + cn],
                            in_=eT[ci][:, 1, N + ro : N + ro + rn],
                            identity=cID[0:cn, 0:cn],
                        )

            def step5(g, ci, eT):
                ro, rn = _CHUNKS[ci]
                for h in range(H):
                    for cc, (co, cn) in enumerate(_CHUNKS):
                        nc.tensor.matmul(
                            oU2[ci][0:rn, g, 65 * h : 65 * h + 65],
                            lhsT=eT[cc][
                                :, h % 2,
                                199 * (h // 2) + ro : 199 * (h // 2) + ro + rn,
                            ],
                            rhs=cXTall[0:cn, b0 + g, cc, 65 * h : 65 * h + 65],
                            start=(cc == 0),
                            stop=(cc == 1),
                        )

            transposes(0, eB0)
            step5(0, 0, eB0)
            step5(0, 1, eB0)
            # pipelined step1 for the NEXT pair sits here: PE-independent of
            # this pair's remaining exps, fills the wait for exp(b1).
            if b0 + 2 < bpc:
                if (b0 + 2) % BG == 0:
                    xg = load_xgroup(b0 + 2)
                fs_next = step1(xg, b0 + 2, q + 1)
            transposes(1, eB1)
            step5(1, 0, eB1)
            step5(1, 1, eB1)

            for ci, (ro, rn) in enumerate(_CHUNKS):
                oUF2 = oU2[ci]
                # 1/s for both items x 4 heads: s at col 64 of each 65-block
                rec = rpool.tile([rn, 2, H], DT, tag=f"rec{ci}")
                oUh = oUF2[0:rn, :, 0 : H * FA].rearrange(
                    "p g (h j) -> p g h j", j=FA
                )
                nc.vector.reciprocal(rec[:], oUh[:, :, :, 64])

                sO2 = sog[ci][:, go : go + 2]
                nc.vector.tensor_tensor(
                    sO2[:, :, 0 : H * U].rearrange("p g (h u) -> p g h u", u=U),
                    oUh[:, :, :, 0:U],
                    rec[:, :, :, None].to_broadcast((rn, 2, H, U)),
                    ALU.mult,
                )
                # mask3 normalize as per-item ACT scale-copies: these have no
                # fan-in until the pair's very end, so they execute in the
                # ACT gap at the next pair's start (exp waits on dense there).
                if os.environ.get("M3E", "vector") == "scalar":
                    for g in range(2):
                        nc.scalar.activation(
                            sO2[:, g, H * U : OUTC],
                            pRv[ci][0:rn, g, 520 : 520 + N],
                            AF.Copy,
                            scale=rec[:, g, 3:4],
                        )
                else:
                    nc.vector.tensor_tensor(
                        sO2[:, :, H * U : OUTC],
                        pRv[ci][0:rn, :, 520 : 520 + N],
                        rec[:, :, 3:4].to_broadcast((rn, 2, N)),
                        ALU.mult,
                    )

                if (go + 2 == OG or b0 + 2 >= bpc) and (
                    os.environ.get("SKIPO", "0") != "1"
                ):
                    # O is laid out (group, N, OG, OUTC) so each partition row
                    # writes one contiguous OG*OUTC*2-byte run; the host
                    # un-permutes the (group, OG) split afterwards.
                    oqm = os.environ.get("OQ2", "1")
                    if oqm == "3q":
                        oq = nc.scalar if ci == 0 else nc.gpsimd
                    elif oqm == "1" and ci == 1:
                        oq = nc.gpsimd
                    else:
                        oq = nc.sync
                    oq.dma_start(O[b0 // OG, ro : ro + rn], sog[ci][:])


def build_nc(
    bpc: int = BPC, num_devices: int = NCORES, repeat: int = 1
) -> bass.Bass:
    nc = bacc.Bacc(
        "TRN2",
        target_bir_lowering=False,
        debug=False,
        num_devices=num_devices,
    )
    io = {
        "Xf": nc.dram_tensor("Xf", [bpc, N, F], BF, kind="ExternalInput").ap(),
        "XaT": nc.dram_tensor("XaT", [bpc, FA, N], BF, kind="ExternalInput").ap(),
        "AWp": nc.dram_tensor("AWp", [N, 2, 2 * N], BF, kind="ExternalInput").ap(),
        "K2": nc.dram_tensor("K2", [128, H, N], BF, kind="ExternalInput").ap(),
        "MK8": nc.dram_tensor("MK8", [N, 2, 2, 2 * N], F8, kind="ExternalInput").ap(),
        "ID8": nc.dram_tensor("ID8", [128, 2, 128], F8, kind="ExternalInput").ap(),
        "TA": nc.dram_tensor("TA", [FA, H, FA], BF, kind="ExternalInput").ap(),
        "ID": nc.dram_tensor("ID", [128, 128], BF, kind="ExternalInput").ap(),
        "O": nc.dram_tensor(
            "O", [bpc // OGRP, N, OGRP, OUTC], BF, kind="ExternalOutput"
        ).ap(),
    }
    with tile.TileContext(nc) as tc:
        if repeat == 1:
            _build_kernel_v3(nc, tc, io, bpc=bpc)
        else:
            # Timing-only variant: re-run the identical workload `repeat`
            # times in a hardware loop so per-dispatch tunnel latency can be
            # amortized out of the hardware-time measurement.
            import os as _os

            if _os.environ.get("STAGR", "1") == "1":
                with tc.For_i(0, repeat, 1, staggered_reset=True):
                    _build_kernel_v3(nc, tc, io, bpc=bpc)
            else:
                with tc.For_i(0, repeat, 1):
                    _build_kernel_v3(nc, tc, io, bpc=bpc)
    nc.compile()
    return nc


def _prep_weights(A, W, kernel, T, bias1, bias2):
    """Host-side constant prep (tiny tensors)."""
    A = np.asarray(A, np.float32)
    W = np.asarray(W, np.float32)
    kernel = np.asarray(kernel, np.float32)
    T = np.asarray(T, np.float32)
    bias1 = np.asarray(bias1, np.float32)
    bias2 = np.asarray(bias2, np.float32)

    AW = A[None, :, :] * W  # (H, n, m)
    # AWp[n, hp, k*199+m] = AW[2hp+k][n, m]
    AWp = np.ascontiguousarray(
        AW.reshape(2, 2, N, N).transpose(2, 0, 1, 3).reshape(N, 2, 2 * N)
    )

    Kf = kernel  # (H, F, N): [h, f, c]
    K1 = np.ascontiguousarray(Kf.transpose(1, 0, 2))  # [f, h, c]
    K2 = np.concatenate([K1, K1], axis=0)  # duplicate f-rows for PE rows 64-127

    # MK[c, h, m] = bias1[h, c] - 60 * (1 - A[m, c]): additive logit fixup
    # (adjacency mask + bias1); -60 and 0 are exactly representable in fp8e4.
    # Packed as [c, s, ktile, k*199 + m] with h = 2k + s; ktile 1 is zeros
    # (the second DoubleRow contraction tile contributes nothing).
    MK = bias1.T[:, :, None] - 60.0 * (1.0 - A.T[:, None, :])  # (c, h, m)
    MKs = MK.reshape(N, 2, 2, N).transpose(0, 2, 1, 3).reshape(N, 2, 2 * N)
    MK8 = np.zeros((N, 2, 2, 2 * N), np.float32)
    MK8[:, :, 0, :] = MKs

    # T_aug[h]: (65, 65): rows 0-63 = T[h], row 64 = [bias2[h], 1.0-at-col-64]
    TA = np.zeros((FA, H, FA), np.float32)
    TA[:F, :, :U] = T.transpose(1, 0, 2)
    TA[F, :, :U] = bias2
    TA[F, :, U] = 1.0

    ID8 = np.zeros((128, 2, 128), np.float32)
    ID8[:, 0, :] = np.eye(128, dtype=np.float32)

    import ml_dtypes

    bf = ml_dtypes.bfloat16
    f8 = ml_dtypes.float8_e4m3
    return dict(
        AWp=AWp.astype(bf), K2=K2.astype(bf), MK8=MK8.astype(f8),
        ID8=ID8.astype(f8), TA=TA.astype(bf), ID=np.eye(128, dtype=bf),
    )


_CACHED = {}


def _get_executable(repeat: int = 1):
    """Build the Bass module once and wrap it in a reusable sharded jax jit.

    Mirrors concourse.bass2jax.run_bass_via_pjrt's multi-core path, but caches
    the jitted callable so repeated kernel() calls skip re-lowering the BIR.
    """
    if repeat in _CACHED:
        return _CACHED[repeat]

    import jax
    from jax.sharding import Mesh, PartitionSpec
    from jax.experimental.shard_map import shard_map

    import concourse.mybir as _mybir
    from concourse import bass2jax

    bass2jax.install_neuronx_cc_hook()
    nc = build_nc(repeat=repeat)

    partition_name = (
        nc.partition_id_tensor.name if nc.partition_id_tensor else None
    )
    in_names, out_names, out_avals = [], [], []
    for alloc in nc.m.functions[0].allocations:
        if not isinstance(alloc, _mybir.MemoryLocationSet):
            continue
        name = alloc.memorylocations[0].name
        if alloc.kind == "ExternalInput":
            if name != partition_name:
                in_names.append(name)
        elif alloc.kind == "ExternalOutput":
            out_names.append(name)
            out_avals.append(
                jax.core.ShapedArray(
                    tuple(alloc.tensor_shape), _mybir.dt.np(alloc.dtype)
                )
            )
    n_params = len(in_names)
    n_outs = len(out_avals)
    all_in_names = list(in_names) + list(out_names)
    if partition_name is not None:
        all_in_names.append(partition_name)

    def _body(*args):
        operands = list(args)
        if partition_name is not None:
            operands.append(bass2jax.partition_id_tensor())
        outs = bass2jax._bass_exec_p.bind(
            *operands,
            out_avals=tuple(out_avals),
            in_names=tuple(all_in_names),
            out_names=tuple(out_names),
            lowering_input_output_aliases=(),
            sim_require_finite=True,
            sim_require_nnan=True,
            nc=nc,
        )
        return tuple(outs)

    devices = jax.devices()[:NCORES]
    mesh = Mesh(np.asarray(devices), ("core",))
    in_specs = (PartitionSpec("core"),) * (n_params + n_outs)
    out_specs = (PartitionSpec("core"),) * n_outs
    sharded = jax.jit(
        shard_map(
            _body, mesh=mesh, in_specs=in_specs, out_specs=out_specs,
            check_rep=False,
        ),
        donate_argnums=tuple(range(n_params, n_params + n_outs)),
        keep_unused=True,
    )
    _CACHED[repeat] = (sharded, in_names, out_names, out_avals, jax, mesh)
    return _CACHED[repeat]


def _stage_inputs(inputs):
    import ml_dtypes

    X = np.asarray(inputs["X"], np.float32)
    consts = _prep_weights(
        inputs["A"], inputs["W"], inputs["kernel"], inputs["T"],
        inputs["bias1"], inputs["bias2"],
    )
    bf = ml_dtypes.bfloat16
    Xb = X.astype(bf)
    XaT = np.concatenate(
        [X.transpose(0, 2, 1), np.ones((B, 1, N), np.float32)], axis=1
    ).astype(bf)
    per_core = {
        "Xf": np.ascontiguousarray(Xb),
        "XaT": np.ascontiguousarray(XaT),
    }
    for k, v in consts.items():
        per_core[k] = np.concatenate([v] * NCORES, axis=0)
    return per_core


def _run(staged):
    sharded, in_names, out_names, out_avals, jax, mesh = _get_executable()
    concat_in = [staged[nm] for nm in in_names]
    zeros = [
        np.zeros((NCORES * a.shape[0], *a.shape[1:]), a.dtype) for a in out_avals
    ]
    out_arrs = sharded(*concat_in, *zeros)
    return np.asarray(out_arrs[out_names.index("O")])


def kernel(**inputs) -> np.ndarray:
    staged = _stage_inputs(inputs)
    out = _run(staged)  # (NCORES*(BPC//OGRP), N, OGRP, OUTC) bf16
    out = out.astype(np.float32)
    out = out.reshape(NCORES, BPC // OGRP, N, OGRP, OUTC)
    out = out.transpose(0, 1, 3, 2, 4).reshape(B, N, OUTC)
    return np.ascontiguousarray(out)


# revision 27
# speedup vs baseline: 1.3301x; 1.0131x over previous
"""Trainium2 Bass kernel for DGCRNNCell (nn_DGCRNNCell_21792664060192).

Computes, for each batch item b and head h over graph with N=199 nodes:
  feat   = einsum('nf,nm->mf', X[b], A*W[h])          (via featT chain)
  dense  = feat @ kernel[h] + bias1[h]
  mask   = softmax(dense - NEG*(1-A), axis=-1)        (adjacency-masked softmax)
  node   = mask @ X[b]
  out_h  = node @ T[h] + bias2[h]
  output[b] = concat([out_0..out_3 (r, 256)], mask_3 (r, 199))   -> (199, 455)

Sharding: pure data-parallel over batch (512 -> 64 per core x 8 cores).

v3 dataflow (per core), built around item PAIRS and engine balance
(GPSIMD cannot touch PSUM on TRN2, so all PSUM-side elementwise work is
split between ACT and DVE with as few, as large instructions as possible):
  step1  featT for a pair (b0,b1): lhsT = [X[b0] | X[b1]] (cn, 128) so the
         pair's f-rows land on partitions 0-63 / 64-127; rhs = AW head-pair
         (cn, 2*199).  4 matmuls of free 398 per pair.
  fs     PSUM->SBUF bf16 copy of the pair's featT; alternates ACT/DVE.
  dense  per item: adjacency mask + bias1 written via an fp8 DoubleRow
         identity matmul (half cycles; -60/0 are exact in fp8e4), then 8
         bf16 matmuls accumulate kernel[h]^T @ featT; exp on ACT gives the
         masked e directly (one activation per c-chunk).
  XT     per item: X_aug @ T_aug, ONE matmul per c-chunk (free 260);
         col 64 of each head block = ones -> s; TA row 64 = bias2.
  step5  out = (e_h)^T @ XT_h accumulated over c-chunks into a 2-item PSUM
         tile; head-3 mask via PE transpose of e3 into a 2-item PSUM tile.
  stage  per item pair: ONE reciprocal, ONE normalize-multiply (512 free)
         and ONE mask3-multiply (398 free) on DVE; output staged bf16
         (host casts to fp32).
PSUM budget (8 banks): ring{fAB,d0,d1,XT} 2x2 banks; oUF2 2 banks;
pR2 2x1 banks.
"""

import os as _os

import numpy as np

import concourse.bass as bass
import concourse.mybir as mybir
import concourse.tile as tile
from concourse import bacc

if _os.environ.get("LDWOPT", "0") == "1":
    # The PE spends real time reloading stationary weights before every
    # matmul; walrus's ldweights-dedup optimization (its own default) is
    # pinned off by bass's production caller — turn it back on for this
    # kernel's compile.
    import concourse.bass_utils as _bu

    if not getattr(_bu, "_ldwopt_patched", False):
        _orig_run_command = _bu.run_command

        def _run_command_ldwopt(argv, **kwargs):
            argv = [
                "--enable-ldw-opt=true" if a == "--enable-ldw-opt=false" else a
                for a in argv
            ]
            return _orig_run_command(argv, **kwargs)

        _bu.run_command = _run_command_ldwopt
        _bu._ldwopt_patched = True

B, N, F, U, H = 512, 199, 64, 64, 4
NCORES = 8
BPC = B // NCORES  # 64 batch items per core
P0 = 128
P1 = N - P0  # 71
FA = F + 1  # X augmented with ones column (XaT row 64 = ones)
OUTC = H * U + N  # 455
DT = mybir.dt.float32
BF = mybir.dt.bfloat16
F8 = mybir.dt.float8e4
AF = mybir.ActivationFunctionType
ALU = mybir.AluOpType
PM = mybir.MatmulPerfMode

_CHUNKS = ((0, P0), (P0, P1))  # (offset, size) along the N(=c or r) axis
OGRP = 4  # output DMA item-group (must match the OG default below)


def _build_kernel_v3(nc: bass.Bass, tc: "tile.TileContext", io: dict, bpc: int = BPC):
    import os
    from contextlib import ExitStack

    Xf, XaT, AWp, K2, MK8, ID8, TA, ID, O = (
        io["Xf"], io["XaT"], io["AWp"], io["K2"], io["MK8"], io["ID8"],
        io["TA"], io["ID"], io["O"],
    )

    def _b(name, default):
        return int(os.environ.get(name, str(default)))

    fse = os.environ.get("FSE", "alt")     # fs copy engine: alt|scalar|vector
    mask8 = os.environ.get("MASK8", "1") == "1"  # fp8 DoubleRow mask write

    with ExitStack() as ctx:
        cpool = ctx.enter_context(tc.tile_pool(name="consts", bufs=1))
        xpool = ctx.enter_context(tc.tile_pool(name="xf", bufs=_b("XB", 2)))
        fspool = ctx.enter_context(tc.tile_pool(name="fs", bufs=_b("FSB", 2)))
        epool = ctx.enter_context(tc.tile_pool(name="expT", bufs=_b("EB", 3)))
        rpool = ctx.enter_context(tc.tile_pool(name="rec", bufs=_b("RB", 4)))
        opool = ctx.enter_context(tc.tile_pool(name="sO", bufs=_b("OB", 2)))

        # ---- constants into SBUF (once) ----
        cAW = []
        cMK = []
        for ci, (co, cn) in enumerate(_CHUNKS):
            t = cpool.tile([cn, 2, 2 * N], BF, name=f"cAW{ci}")
            nc.sync.dma_start(t[:], AWp[co : co + cn])
            cAW.append(t)
            if mask8:
                t = cpool.tile([cn, 2, 2, 2 * N], F8, name=f"cMK{ci}")
                nc.sync.dma_start(t[:], MK8[co : co + cn])
                cMK.append(t)
        cK2 = cpool.tile([128, H, N], BF, name="cK2")
        cTA = cpool.tile([FA, H, FA], BF, name="cTA")
        cID = cpool.tile([128, 128], BF, name="cID")
        nc.sync.dma_start(cK2[:], K2[:])
        nc.sync.dma_start(cTA[:], TA[:])
        nc.sync.dma_start(cID[:], ID[:])
        if mask8:
            cID8 = cpool.tile([128, 2, 128], F8, name="cID8")
            nc.sync.dma_start(cID8[:], ID8[:])

        BG = min(_b("BG", 8), bpc)   # input DMA batching
        OG = OGRP  # output DMA batching (matches the DRAM layout)

        # ---- prologue: XT = Xa_aug @ TA_aug for ALL items, kept in SBUF ----
        # cXTall[c, b, cc, 65h + j]: cols 0-63 of each head block = XT data,
        # col 64 = ones (the softmax-denominator column), written once.
        cXTall = cpool.tile([128, bpc, 2, H * FA], BF, name="cXTall")
        nc.vector.memset(
            cXTall[:].rearrange("p b c (h j) -> p b c h j", j=FA)[:, :, :, :, 64],
            1.0,
        )
        PG = 4  # items per prologue PSUM tile
        with tc.tile_pool(name="pxt", bufs=_b("XTB", 2), space="PSUM") as pxt:
            for b0 in range(0, bpc, PG):
                if b0 % BG == 0:
                    ng = min(BG, bpc - b0)
                    xtg = xpool.tile([FA, BG * N], BF, tag="xat")
                    nc.sync.dma_start(
                        xtg[:, 0 : ng * N].rearrange("j (g n) -> j g n", n=N),
                        XaT[b0 : b0 + ng].rearrange("g j n -> j g n"),
                    )
                gi = b0 % BG
                XTp = pxt.tile([128, PG, 2, 256], DT, tag="xtp")
                for g in range(PG):
                    xt = xtg[:, (gi + g) * N : (gi + g + 1) * N]
                    for ci, (co, cn) in enumerate(_CHUNKS):
                        nc.tensor.matmul(
                            XTp[0:cn, g, ci, :],
                            lhsT=xt[:, co : co + cn],
                            rhs=cTA[:, :, 0:U],
                            start=True,
                            stop=True,
                        )
                dst = cXTall[:, b0 : b0 + PG].rearrange(
                    "p b c (h j) -> p b c h j", j=FA
                )[:, :, :, :, 0:U]
                if (b0 // PG) % 2 == 0:
                    nc.scalar.copy(
                        dst, XTp[:].rearrange("p b c (h u) -> p b c h u", u=U)
                    )
                else:
                    nc.vector.tensor_copy(
                        dst, XTp[:].rearrange("p b c (h u) -> p b c h u", u=U)
                    )

        pd = ctx.enter_context(
            tc.tile_pool(name="pdnu", bufs=_b("DTB", 2), space="PSUM")
        )
        po = ctx.enter_context(
            tc.tile_pool(name="poU", bufs=_b("POB", 2), space="PSUM")
        )

        def load_xgroup(b0):
            ng = min(BG, bpc - b0)
            src = Xf[b0 : b0 + ng].rearrange("g n f -> n g f")
            xg = []
            for ci, (co, cn) in enumerate(_CHUNKS):
                t = xpool.tile([cn, BG, F], BF, tag=f"xf{ci}")
                nc.sync.dma_start(t[:, 0:ng, :], src[co : co + cn])
                xg.append(t)
            return xg

        def step1(xg, b0, q):
            # step1 for a pair: out partitions = [b0 f | b1 f].  Allocated
            # from the dnu ring; emitted one pair AHEAD so the copy+dense of
            # pair q+1 overlap the exp drain of pair q.
            gi = b0 % BG
            fAB = pd.tile([128, 2, 512], DT, tag="dnu", name="fAB")
            for hp in range(2):
                for ci, (co, cn) in enumerate(_CHUNKS):
                    nc.tensor.matmul(
                        fAB[:, hp, 0 : 2 * N],
                        lhsT=xg[ci][:, gi : gi + 2, :],
                        rhs=cAW[ci][:, hp, :],
                        start=(ci == 0),
                        stop=(ci == 1),
                    )
            fs = fspool.tile([128, 2, 2 * N], BF, tag="fs")
            if fse == "split":
                nc.scalar.copy(fs[:, 0], fAB[:, 0, 0 : 2 * N])
                nc.vector.tensor_copy(fs[:, 1], fAB[:, 1, 0 : 2 * N])
            elif fse == "scalar" or (fse == "alt" and q % 2 == 0):
                nc.scalar.copy(fs[:], fAB[:, :, 0 : 2 * N])
            else:
                nc.vector.tensor_copy(fs[:], fAB[:, :, 0 : 2 * N])
            return fs

        xg = load_xgroup(0)
        fs_next = step1(xg, 0, 0)
        sog = [None, None]
        for q in range(bpc // 2):
            b0 = 2 * q
            fs = fs_next

            def dense_chunk(g, ci, pool):
                # dense chunk + exp; head h -> slot s=h%2, block k=h//2
                co, cn = _CHUNKS[ci]
                d = pool.tile([128, 2, 512], DT,
                              tag="dnu" if pool is pd else "oU",
                              name=f"dT{g}{ci}")
                if mask8 and os.environ.get("MASKOFF", "0") != "1":
                    for s in range(2):
                        nc.tensor.matmul(
                            d[0:cn, s, 0 : 2 * N],
                            lhsT=cID8[0:cn, :, 0:cn],
                            rhs=cMK[ci][:, s],
                            start=True,
                            stop=False,
                            perf_mode=PM.DoubleRow,
                        )
                for h in range(H):
                    nc.tensor.matmul(
                        d[0:cn, h % 2, 199 * (h // 2) : 199 * (h // 2) + N],
                        lhsT=cK2[64 * g : 64 * g + 64, h, co : co + cn],
                        rhs=fs[64 * g : 64 * g + 64, h // 2,
                               199 * (h % 2) : 199 * (h % 2) + N],
                        start=(not mask8
                               or os.environ.get("MASKOFF", "0") == "1"),
                        stop=True,
                        tile_position=(64 * g, 0),
                    )
                e = epool.tile([cn, 2, 2 * N], BF, tag=f"eT{ci}")
                nc.scalar.activation(e[:], d[0:cn, :, 0 : 2 * N], AF.Exp)
                return e

            go = b0 % OG
            if go == 0:
                sog = [
                    opool.tile([rn, OG, OUTC], BF, tag=f"sO{ci}", name=f"sOg{ci}")
                    for ci, (ro, rn) in enumerate(_CHUNKS)
                ]

            # item 0's dense through the dnu ring, item 1's through the oU
            # ring: the next pair's step1 then reuses the buffer freed by the
            # FIRST exp, so it overlaps the tail of this pair's exp drain.
            dp1 = po if os.environ.get("D1P", "0") == "1" else pd
            eB0 = [dense_chunk(0, 0, pd), dense_chunk(0, 1, pd)]
            eB1 = [dense_chunk(1, 0, dp1), dense_chunk(1, 1, dp1)]
            eTg = [eB0, eB1]

            oU2 = [po.tile([128, 2, 512], DT, tag="oU", name=f"oUF{ci}")
                   for ci in range(2)]
            # head-3 mask lives (as bf16) in the padding of the oU2 slots:
            # slot bytes [0:1040) hold the 260-col step5 output, [1040:1840)
            # hold the transposed e3 row-chunk for the same r-range.
            pRv = [oU2[ci][:].bitcast(BF) for ci in range(2)]

            def transposes(g, eT):
                # head-3 mask transposed into (r, c): PE-transpose of e3
                for rj, (ro, rn) in enumerate(_CHUNKS):
                    for ci, (co, cn) in enumerate(_CHUNKS):
                        nc.tensor.transpose(
                            pRv[rj][0:rn, g, 520 + co : 520 + co + cn],
                            in_=eT[ci][:, 1, N + ro : N + ro + rn],
                            identity=cID[0:cn, 0:cn],
                        )

            def step5(g, ci, eT):
                ro, rn = _CHUNKS[ci]
                for h in range(H):
                    for cc, (co, cn) in enumerate(_CHUNKS):
                        nc.tensor.matmul(
                            oU2[ci][0:rn, g, 65 * h : 65 * h + 65],
                            lhsT=eT[cc][
                                :, h % 2,
                                199 * (h // 2) + ro : 199 * (h // 2) + ro + rn,
                            ],
                            rhs=cXTall[0:cn, b0 + g, cc, 65 * h : 65 * h + 65],
                            start=(cc == 0),
                            stop=(cc == 1),
                        )

            transposes(0, eB0)
            step5(0, 0, eB0)
            step5(0, 1, eB0)
            # pipelined step1 for the NEXT pair sits here: PE-independent of
            # this pair's remaining exps, fills the wait for exp(b1).
            if b0 + 2 < bpc:
                if (b0 + 2) % BG == 0:
                    xg = load_xgroup(b0 + 2)
                fs_next = step1(xg, b0 + 2, q + 1)
            transposes(1, eB1)
            step5(1, 0, eB1)
            step5(1, 1, eB1)

            for ci, (ro, rn) in enumerate(_CHUNKS):
                oUF2 = oU2[ci]
                # 1/s for both items x 4 heads: s at col 64 of each 65-block
                rec = rpool.tile([rn, 2, H], DT, tag=f"rec{ci}")
                oUh = oUF2[0:rn, :, 0 : H * FA].rearrange(
                    "p g (h j) -> p g h j", j=FA
                )
                nc.vector.reciprocal(rec[:], oUh[:, :, :, 64])

                sO2 = sog[ci][:, go : go + 2]
                nc.vector.tensor_tensor(
                    sO2[:, :, 0 : H * U].rearrange("p g (h u) -> p g h u", u=U),
                    oUh[:, :, :, 0:U],
                    rec[:, :, :, None].to_broadcast((rn, 2, H, U)),
                    ALU.mult,
                )
                # mask3 normalize as per-item ACT scale-copies: these have no
                # fan-in until the pair's very end, so they execute in the
                # ACT gap at the next pair's start (exp waits on dense there).
                if os.environ.get("M3E", "vector") == "scalar":
                    for g in range(2):
                        nc.scalar.activation(
                            sO2[:, g, H * U : OUTC],
                            pRv[ci][0:rn, g, 520 : 520 + N],
                            AF.Copy,
                            scale=rec[:, g, 3:4],
                        )
                else:
                    nc.vector.tensor_tensor(
                        sO2[:, :, H * U : OUTC],
                        pRv[ci][0:rn, :, 520 : 520 + N],
                        rec[:, :, 3:4].to_broadcast((rn, 2, N)),
                        ALU.mult,
                    )

                if (go + 2 == OG or b0 + 2 >= bpc) and (
                    os.environ.get("SKIPO", "0") != "1"
                ):
                    # O is laid out (group, N, OG, OUTC) so each partition row
                    # writes one contiguous OG*OUTC*2-byte run; the host
                    # un-permutes the (group, OG) split afterwards.
                    oqm = os.environ.get("OQ2", "1")
                    if oqm == "3q":
                        oq = nc.scalar if ci == 0 else nc.gpsimd
                    elif oqm == "all":
                        oq = nc.gpsimd
                    elif oqm == "1" and ci == 1:
                        oq = nc.gpsimd
                    else:
                        oq = nc.sync
                    oq.dma_start(O[b0 // OG, ro : ro + rn], sog[ci][:])


def build_nc(
    bpc: int = BPC, num_devices: int = NCORES, repeat: int = 1
) -> bass.Bass:
    nc = bacc.Bacc(
        "TRN2",
        target_bir_lowering=False,
        debug=False,
        num_devices=num_devices,
    )
    io = {
        "Xf": nc.dram_tensor("Xf", [bpc, N, F], BF, kind="ExternalInput").ap(),
        "XaT": nc.dram_tensor("XaT", [bpc, FA, N], BF, kind="ExternalInput").ap(),
        "AWp": nc.dram_tensor("AWp", [N, 2, 2 * N], BF, kind="ExternalInput").ap(),
        "K2": nc.dram_tensor("K2", [128, H, N], BF, kind="ExternalInput").ap(),
        "MK8": nc.dram_tensor("MK8", [N, 2, 2, 2 * N], F8, kind="ExternalInput").ap(),
        "ID8": nc.dram_tensor("ID8", [128, 2, 128], F8, kind="ExternalInput").ap(),
        "TA": nc.dram_tensor("TA", [FA, H, FA], BF, kind="ExternalInput").ap(),
        "ID": nc.dram_tensor("ID", [128, 128], BF, kind="ExternalInput").ap(),
        "O": nc.dram_tensor(
            "O", [bpc // OGRP, N, OGRP, OUTC], BF, kind="ExternalOutput"
        ).ap(),
    }
    with tile.TileContext(nc) as tc:
        if repeat == 1:
            _build_kernel_v3(nc, tc, io, bpc=bpc)
        else:
            # Timing-only variant: re-run the identical workload `repeat`
            # times in a hardware loop so per-dispatch tunnel latency can be
            # amortized out of the hardware-time measurement.
            import os as _os

            if _os.environ.get("STAGR", "1") == "1":
                with tc.For_i(0, repeat, 1, staggered_reset=True):
                    _build_kernel_v3(nc, tc, io, bpc=bpc)
            else:
                with tc.For_i(0, repeat, 1):
                    _build_kernel_v3(nc, tc, io, bpc=bpc)
    nc.compile()
    return nc


def _prep_weights(A, W, kernel, T, bias1, bias2):
    """Host-side constant prep (tiny tensors)."""
    A = np.asarray(A, np.float32)
    W = np.asarray(W, np.float32)
    kernel = np.asarray(kernel, np.float32)
    T = np.asarray(T, np.float32)
    bias1 = np.asarray(bias1, np.float32)
    bias2 = np.asarray(bias2, np.float32)

    AW = A[None, :, :] * W  # (H, n, m)
    # AWp[n, hp, k*199+m] = AW[2hp+k][n, m]
    AWp = np.ascontiguousarray(
        AW.reshape(2, 2, N, N).transpose(2, 0, 1, 3).reshape(N, 2, 2 * N)
    )

    Kf = kernel  # (H, F, N): [h, f, c]
    K1 = np.ascontiguousarray(Kf.transpose(1, 0, 2))  # [f, h, c]
    K2 = np.concatenate([K1, K1], axis=0)  # duplicate f-rows for PE rows 64-127

    # MK[c, h, m] = bias1[h, c] - 60 * (1 - A[m, c]): additive logit fixup
    # (adjacency mask + bias1); -60 and 0 are exactly representable in fp8e4.
    # Packed as [c, s, ktile, k*199 + m] with h = 2k + s; ktile 1 is zeros
    # (the second DoubleRow contraction tile contributes nothing).
    MK = bias1.T[:, :, None] - 60.0 * (1.0 - A.T[:, None, :])  # (c, h, m)
    MKs = MK.reshape(N, 2, 2, N).transpose(0, 2, 1, 3).reshape(N, 2, 2 * N)
    MK8 = np.zeros((N, 2, 2, 2 * N), np.float32)
    MK8[:, :, 0, :] = MKs

    # T_aug[h]: (65, 65): rows 0-63 = T[h], row 64 = [bias2[h], 1.0-at-col-64]
    TA = np.zeros((FA, H, FA), np.float32)
    TA[:F, :, :U] = T.transpose(1, 0, 2)
    TA[F, :, :U] = bias2
    TA[F, :, U] = 1.0

    ID8 = np.zeros((128, 2, 128), np.float32)
    ID8[:, 0, :] = np.eye(128, dtype=np.float32)

    import ml_dtypes

    bf = ml_dtypes.bfloat16
    f8 = ml_dtypes.float8_e4m3
    return dict(
        AWp=AWp.astype(bf), K2=K2.astype(bf), MK8=MK8.astype(f8),
        ID8=ID8.astype(f8), TA=TA.astype(bf), ID=np.eye(128, dtype=bf),
    )


_CACHED = {}


def _get_executable(repeat: int = 1):
    """Build the Bass module once and wrap it in a reusable sharded jax jit.

    Mirrors concourse.bass2jax.run_bass_via_pjrt's multi-core path, but caches
    the jitted callable so repeated kernel() calls skip re-lowering the BIR.
    """
    if repeat in _CACHED:
        return _CACHED[repeat]

    import jax
    from jax.sharding import Mesh, PartitionSpec
    from jax.experimental.shard_map import shard_map

    import concourse.mybir as _mybir
    from concourse import bass2jax

    bass2jax.install_neuronx_cc_hook()
    nc = build_nc(repeat=repeat)

    partition_name = (
        nc.partition_id_tensor.name if nc.partition_id_tensor else None
    )
    in_names, out_names, out_avals = [], [], []
    for alloc in nc.m.functions[0].allocations:
        if not isinstance(alloc, _mybir.MemoryLocationSet):
            continue
        name = alloc.memorylocations[0].name
        if alloc.kind == "ExternalInput":
            if name != partition_name:
                in_names.append(name)
        elif alloc.kind == "ExternalOutput":
            out_names.append(name)
            out_avals.append(
                jax.core.ShapedArray(
                    tuple(alloc.tensor_shape), _mybir.dt.np(alloc.dtype)
                )
            )
    n_params = len(in_names)
    n_outs = len(out_avals)
    all_in_names = list(in_names) + list(out_names)
    if partition_name is not None:
        all_in_names.append(partition_name)

    def _body(*args):
        operands = list(args)
        if partition_name is not None:
            operands.append(bass2jax.partition_id_tensor())
        outs = bass2jax._bass_exec_p.bind(
            *operands,
            out_avals=tuple(out_avals),
            in_names=tuple(all_in_names),
            out_names=tuple(out_names),
            lowering_input_output_aliases=(),
            sim_require_finite=True,
            sim_require_nnan=True,
            nc=nc,
        )
        return tuple(outs)

    devices = jax.devices()[:NCORES]
    mesh = Mesh(np.asarray(devices), ("core",))
    in_specs = (PartitionSpec("core"),) * (n_params + n_outs)
    out_specs = (PartitionSpec("core"),) * n_outs
    sharded = jax.jit(
        shard_map(
            _body, mesh=mesh, in_specs=in_specs, out_specs=out_specs,
            check_rep=False,
        ),
        donate_argnums=tuple(range(n_params, n_params + n_outs)),
        keep_unused=True,
    )
    _CACHED[repeat] = (sharded, in_names, out_names, out_avals, jax, mesh)
    return _CACHED[repeat]


def _stage_inputs(inputs):
    import ml_dtypes

    X = np.asarray(inputs["X"], np.float32)
    consts = _prep_weights(
        inputs["A"], inputs["W"], inputs["kernel"], inputs["T"],
        inputs["bias1"], inputs["bias2"],
    )
    bf = ml_dtypes.bfloat16
    Xb = X.astype(bf)
    XaT = np.concatenate(
        [X.transpose(0, 2, 1), np.ones((B, 1, N), np.float32)], axis=1
    ).astype(bf)
    per_core = {
        "Xf": np.ascontiguousarray(Xb),
        "XaT": np.ascontiguousarray(XaT),
    }
    for k, v in consts.items():
        per_core[k] = np.concatenate([v] * NCORES, axis=0)
    return per_core


def _run(staged):
    sharded, in_names, out_names, out_avals, jax, mesh = _get_executable()
    concat_in = [staged[nm] for nm in in_names]
    zeros = [
        np.zeros((NCORES * a.shape[0], *a.shape[1:]), a.dtype) for a in out_avals
    ]
    out_arrs = sharded(*concat_in, *zeros)
    return np.asarray(out_arrs[out_names.index("O")])


def kernel(**inputs) -> np.ndarray:
    staged = _stage_inputs(inputs)
    out = _run(staged)  # (NCORES*(BPC//OGRP), N, OGRP, OUTC) bf16
    out = out.astype(np.float32)
    out = out.reshape(NCORES, BPC // OGRP, N, OGRP, OUTC)
    out = out.transpose(0, 1, 3, 2, 4).reshape(B, N, OUTC)
    return np.ascontiguousarray(out)
